# revision 1
# baseline (speedup 1.0000x reference)
"""MixedScoreMultiHeadAttention Trainium2 kernel (PE-centric pipeline).

Data-parallel over batch: 32 batches -> 8 cores x 4 batches.

Per (b):
  dot_h = q_h k_h^T  (per head, PE)  -> flattened r-major into rhs rows
  layer1: T[(h,m), pts] = a[h,m]*dot_h[pts] + c[h,m]*Y[pts]   (PE matmul,
          constant stationary [17,128]; bias b[h,m] folded into relu evac)
  R = relu(T + b)                     (ACT/DVE evacuation from PSUM, fp16)
  layer2: mixed^T[pts, h] via stationary-swapped matmul (lhsT = R data,
          rhs = block-diag w2 [128,8]) -> PSUM [c, (r-grp, h)] full-partition
  exp-evac (ACT Exp) -> w_sb [c, (r,h)] fp32
  AV: out[r, 17] = w^T-slice.T @ [v|1]  (ones col gives softmax denominator)
  normalize by reciprocal of col 16 -> out rows

mix2 bias b2 is dropped (constant shift is softmax-invariant); 1/sqrt(D) is
folded into Wq host-side.
"""
import sys

sys.path.insert(0, "/opt/trn_rl_repo")

import numpy as np
from contextlib import ExitStack

import concourse.bass as bass
import concourse.mybir as mybir
import concourse.tile as tile
from concourse import bacc
from concourse.bass_utils import run_bass_kernel_spmd
from concourse.masks import make_identity

B, R, C, E, H, D, MS = 32, 128, 128, 256, 16, 16, 16
NCORES = 8
BL = B // NCORES  # batches per core: 4
TOK = BL * R      # 512 tokens per core per side
PTS = R * C       # 16384 score points per (b)

FP32 = mybir.dt.float32
FP16 = mybir.dt.float16
AF = mybir.ActivationFunctionType
ALU = mybir.AluOpType



def build_kernel():
    nc = bacc.Bacc("TRN2", target_bir_lowering=False, debug=False,
                   num_devices=NCORES)

    x_r = nc.dram_tensor("x_r", [TOK, E], FP32, kind="ExternalInput").ap()
    x_c = nc.dram_tensor("x_c", [TOK, E], FP32, kind="ExternalInput").ap()
    cost = nc.dram_tensor("cost", [BL, R, C], FP32, kind="ExternalInput").ap()
    # Wq pre-scaled by 1/sqrt(D) host-side; head-padding to 32-col slots
    # (for 32-aligned projection PSUM rows) happens on-chip.
    wq_d = nc.dram_tensor("Wq", [E, E], FP32, kind="ExternalInput").ap()
    wk_d = nc.dram_tensor("Wk", [E, E], FP32, kind="ExternalInput").ap()
    wv_d = nc.dram_tensor("Wv", [E, E], FP32, kind="ExternalInput").ap()
    # layer1 stationary [17, 256]: col (half*128 + (h%8)*16 + m):
    #   row h' = a[h,m] iff h'==h; row 16 = c[h,m]
    w1_d = nc.dram_tensor("W1L", [17, 2 * 128], FP32,
                          kind="ExternalInput").ap()
    # layer2 moving [128, 16]: col (half*8 + j): row hm = w2[half*8+j, m]
    # iff hm == ((j)*16+m) else 0
    w2_d = nc.dram_tensor("W2L", [128, 16], FP32, kind="ExternalInput").ap()
    # relu bias per (h,m) row: bcol2[hm, half] = b1[half*8 + hm//16, hm%16]
    bc_d = nc.dram_tensor("bcol2", [128, 2], FP32, kind="ExternalInput").ap()
    out_d = nc.dram_tensor("out", [BL, R, H * D], FP32,
                           kind="ExternalOutput").ap()

    with tile.TileContext(nc) as tc, ExitStack() as ctx:
        const_p = ctx.enter_context(tc.tile_pool(name="const", bufs=1))
        inx_p = ctx.enter_context(tc.tile_pool(name="inx", bufs=2))
        w_p = ctx.enter_context(tc.tile_pool(name="wts", bufs=1))
        xt_p = ctx.enter_context(tc.tile_pool(name="xt", bufs=1))
        qkv_p = ctx.enter_context(tc.tile_pool(name="qkv", bufs=1))
        x4_p = ctx.enter_context(tc.tile_pool(name="x4", bufs=1))
        rhs_p = ctx.enter_context(tc.tile_pool(name="rhs", bufs=2))
        rr_p = ctx.enter_context(tc.tile_pool(name="rr", bufs=6))
        wsb_p = ctx.enter_context(tc.tile_pool(name="wsb", bufs=2))
        fout_p = ctx.enter_context(tc.tile_pool(name="fout", bufs=1))
        small_p = ctx.enter_context(tc.tile_pool(name="small", bufs=4))
        ps_tr = ctx.enter_context(
            tc.tile_pool(name="pstr", bufs=1, space="PSUM"))
        ps_big = ctx.enter_context(
            tc.tile_pool(name="psb", bufs=4, space="PSUM"))
        ps_l2 = ctx.enter_context(
            tc.tile_pool(name="psl2", bufs=2, space="PSUM"))
        ps_av = ctx.enter_context(
            tc.tile_pool(name="psa", bufs=1, space="PSUM"))

        ident = const_p.tile([128, 128], FP32)
        make_identity(nc, ident[:])

        # ---- small weight/const loads
        w1f = inx_p.tile([17, 2 * 128], FP32, tag="w1f")
        nc.sync.dma_start(w1f[:], w1_d[:])
        w1l = const_p.tile([17, 2 * 128], FP16)
        nc.vector.tensor_copy(w1l[:], w1f[:])

        w2f = inx_p.tile([128, 16], FP32, tag="w2f")
        nc.sync.dma_start(w2f[:], w2_d[:])
        w2l = const_p.tile([128, 16], FP16)
        nc.vector.tensor_copy(w2l[:], w2f[:])

        bcol2 = const_p.tile([128, 2], FP32)
        nc.sync.dma_start(bcol2[:], bc_d[:])

        # ---- QKV weights fp16 (q/k padded on-chip: head h -> 32-col slot)
        wt16 = {}
        for name, dram in (("q", wq_d), ("k", wk_d), ("v", wv_d)):
            halves = []
            for eh in range(2):
                w32 = inx_p.tile([128, E], FP32, tag="wload")
                nc.sync.dma_start(w32[:], dram[eh * 128:(eh + 1) * 128, :])
                ncols = E if name == "v" else 2 * E
                w16 = w_p.tile([128, ncols], FP16, tag=f"w16{name}{eh}",
                               name=f"w16{name}{eh}")
                if name == "v":
                    nc.vector.tensor_copy(w16[:], w32[:])
                else:
                    nc.gpsimd.memset(w16[:], 0.0)
                    w16v = w16[:].rearrange("p (h x) -> p h x", h=H)
                    w32v = w32[:].rearrange("p (h x) -> p h x", h=H)
                    nc.vector.tensor_copy(w16v[:, :, 0:D], w32v[:])
                halves.append(w16)
            wt16[name] = halves

        # ---- x load + PE transpose -> xT fp16 [2 e-halves][128, TOK]
        xT = {}
        for name, dram in (("r", x_r), ("c", x_c)):
            xt0 = xt_p.tile([128, TOK], FP16, tag=f"xT{name}0")
            xt1 = xt_p.tile([128, TOK], FP16, tag=f"xT{name}1")
            xT[name] = [xt0, xt1]
            for t in range(BL):
                x32 = inx_p.tile([128, E], FP32, tag="xload")
                nc.sync.dma_start(x32[:], dram[t * 128:(t + 1) * 128, :])
                for eh in range(2):
                    pst = ps_tr.tile([128, 128], FP32, tag="pstr")
                    nc.tensor.transpose(
                        pst[:], x32[:, eh * 128:(eh + 1) * 128], ident[:])
                    nc.vector.tensor_copy(
                        xT[name][eh][:, t * 128:(t + 1) * 128], pst[:])

        # ---- cost -> fp16 [r, c] tiles (r-major flatten later)
        y16 = []
        for b in range(BL):
            c32 = inx_p.tile([128, C], FP32, tag="cload")
            nc.sync.dma_start(c32[:], cost[b])
            y1 = const_p.tile([128, C], FP16, name=f"y16_{b}", tag=f"y16_{b}")
            nc.vector.tensor_copy(y1[:], c32[:])
            y16.append(y1)

        # ---- projections: qT/kT per-head tiles [16, TOK] fp16
        qT, kT = [], []
        for proj, dst in (("q", qT), ("k", kT)):
            for mh in range(4):  # head-quad tiles (4 heads x 32 rows)
                ps = ps_big.tile([128, TOK], FP32, tag="psbig")
                for eh in range(2):
                    nc.tensor.matmul(
                        ps[:],
                        wt16[proj][eh][:, mh * 128:(mh + 1) * 128],
                        xT["r" if proj == "q" else "c"][eh][:],
                        start=(eh == 0), stop=(eh == 1))
                # pack 3 head evacs in one 96-row op (PE matmul operands may
                # sit at base partition 0/32/64; 96 is invalid -> separate)
                quad = qkv_p.tile([96, TOK], FP16, tag=f"{proj}Q{mh}",
                                  name=f"{proj}Q{mh}")
                last = qkv_p.tile([16, TOK], FP16, tag=f"{proj}L{mh}",
                                  name=f"{proj}L{mh}")
                if mh % 2 == 0:
                    nc.scalar.copy(quad[:], ps[0:96, :])
                    nc.vector.tensor_copy(last[:], ps[96:112, :])
                else:
                    nc.vector.tensor_copy(quad[:], ps[0:96, :])
                    nc.scalar.copy(last[:], ps[96:112, :])
                for hh in range(4):
                    dst.append(quad[hh * 32:hh * 32 + 16, :] if hh < 3
                               else last[:])

        # ---- v natural [c, hd] fp32 interleaved with ones col -> vhat
        vhat = []
        for b in range(BL):
            vh = qkv_p.tile([128, 17 * H], FP32, tag=f"vhat{b}",
                            name=f"vhat{b}")
            vh3 = vh[:].rearrange("p (h x) -> p h x", h=H)
            nc.gpsimd.memset(vh3[:, :, 16:17], 1.0)
            ps = ps_big.tile([128, E], FP32, tag="psbig")
            for eh in range(2):
                nc.tensor.matmul(
                    ps[:], xT["c"][eh][:, b * 128:(b + 1) * 128],
                    wt16["v"][eh][:], start=(eh == 0), stop=(eh == 1))
            nc.scalar.copy(
                vh3[:, :, 0:16], ps[:].rearrange("p (h x) -> p h x", h=H))
            vhat.append(vh)

        # ---- dots: X4[h] fp16 [r, (b, c)]
        x4s = []
        for h in range(H):
            x4 = x4_p.tile([128, BL * C], FP16, tag=f"x4_{h}",
                           name=f"x4_{h}")
            psd = ps_big.tile([128, BL * C], FP32, tag="psbig")
            for b in range(BL):
                nc.tensor.matmul(
                    psd[:, b * 128:(b + 1) * 128],
                    qT[h][:, b * 128:(b + 1) * 128],
                    kT[h][:, b * 128:(b + 1) * 128])
            if h % 2 == 0:
                nc.scalar.copy(x4[:], psd[:])
            else:
                nc.vector.tensor_copy(x4[:], psd[:])
            x4s.append(x4)

        # ---- per (b): layer1+relu, layer2, exp, AV
        fouts = [fout_p.tile([128, H * D], FP32, tag=f"fo{b}", name=f"fo{b}")
                 for b in range(BL)]
        for b in range(BL):
            rhs = rhs_p.tile([17, PTS], FP16, tag="rhs")
            for h in range(H):
                nc.sync.dma_start(rhs[h:h + 1, :],
                                  x4s[h][:, b * 128:(b + 1) * 128])
            nc.sync.dma_start(rhs[16:17, :], y16[b][:])

            for half in range(2):
                wsb = wsb_p.tile([128, 8 * C], FP32)
                for grp in range(2):  # 64 r's per group
                    ps2 = ps_l2.tile([128, 512], FP32)
                    for cki in range(16):  # layer1 chunks of 512 pts
                        ck = grp * 16 + cki
                        rr = rr_p.tile([128, 512], FP16, tag="rr")
                        ps1 = ps_big.tile([128, 512], FP32, tag="psbig")
                        nc.tensor.matmul(
                            ps1[:], w1l[:, half * 128:(half + 1) * 128],
                            rhs[:, ck * 512:(ck + 1) * 512])
                        if ck % 2 == 0:
                            nc.scalar.activation(
                                rr[:], ps1[:], AF.Relu,
                                bias=bcol2[:, half:half + 1])
                        else:
                            nc.vector.tensor_scalar(
                                rr[:], ps1[:], bcol2[:, half:half + 1],
                                0.0, ALU.add, ALU.max)
                        for s in range(4):  # layer2 per 128-pt subchunk
                            rloc = cki * 4 + s
                            nc.tensor.matmul(
                                ps2[:, rloc * 8:rloc * 8 + 8],
                                rr[:, s * 128:(s + 1) * 128],
                                w2l[:, half * 8:(half + 1) * 8])
                    nc.scalar.activation(
                        wsb[:, grp * 512:(grp + 1) * 512], ps2[:], AF.Exp)

                # AV + normalize for the 8 heads of this half
                psa = ps_av.tile([128, 17 * 8], FP32)
                wsb4 = wsb[:].rearrange("p (g s h) -> p g s h", g=2, s=64)
                for hl in range(8):
                    h = half * 8 + hl
                    nc.tensor.matmul(
                        psa[:, hl * 17:(hl + 1) * 17],
                        wsb4[:, :, :, hl],
                        vhat[b][:, h * 17:(h + 1) * 17])
                rec = small_p.tile([128, 8], FP32, tag="rec")
                psa3 = psa[:].rearrange("p (x y) -> p x y", x=8)
                nc.vector.reciprocal(rec[:], psa3[:, :, 16])
                for hl in range(8):
                    h = half * 8 + hl
                    nc.vector.tensor_scalar(
                        fouts[b][:, h * D:(h + 1) * D], psa3[:, hl, 0:16],
                        rec[:, hl:hl + 1], None, ALU.mult)

        for b in range(BL):
            nc.sync.dma_start(out_d[b], fouts[b][:])

    nc.compile()
    return nc


_cache = {}


def kernel(**inputs):
    row_emb = np.asarray(inputs["row_emb"], dtype=np.float32)
    col_emb = np.asarray(inputs["col_emb"], dtype=np.float32)
    cost_mat = np.asarray(inputs["cost_mat"], dtype=np.float32)
    Wq = np.asarray(inputs["Wq"], dtype=np.float32)
    Wk = np.asarray(inputs["Wk"], dtype=np.float32)
    Wv = np.asarray(inputs["Wv"], dtype=np.float32)
    m1w = np.asarray(inputs["mix1_weight"], dtype=np.float32)
    m1b = np.asarray(inputs["mix1_bias"], dtype=np.float32)
    m2w = np.asarray(inputs["mix2_weight"], dtype=np.float32)

    a1 = m1w[:, 0, :]
    c1 = m1w[:, 1, :]
    w2 = m2w[:, :, 0]

    if "nc" not in _cache:
        _cache["nc"] = build_kernel()
    nc = _cache["nc"]

    wq_s = Wq * (1.0 / np.sqrt(D))
    wk_p = Wk

    w1l = np.zeros((17, 256), dtype=np.float32)
    w2l = np.zeros((128, 16), dtype=np.float32)
    bcol2 = np.zeros((128, 2), dtype=np.float32)
    for h in range(H):
        half, hl = h // 8, h % 8
        for m in range(MS):
            col = half * 128 + hl * 16 + m
            w1l[h, col] = a1[h, m]
            w1l[16, col] = c1[h, m]
            w2l[hl * 16 + m, half * 8 + hl] = w2[h, m]
            bcol2[hl * 16 + m, half] = m1b[h, m]

    in_maps = []
    for i in range(NCORES):
        sl = slice(i * BL, (i + 1) * BL)
        in_maps.append({
            "x_r": row_emb[sl].reshape(TOK, E),
            "x_c": col_emb[sl].reshape(TOK, E),
            "cost": cost_mat[sl],
            "Wq": wq_s, "Wk": wk_p, "Wv": Wv,
            "W1L": w1l, "W2L": w2l, "bcol2": bcol2,
        })
    res = run_bass_kernel_spmd(nc, in_maps, list(range(NCORES)))
    out = np.concatenate([res.results[i]["out"] for i in range(NCORES)],
                         axis=0)
    return out.astype(np.float32)



# revision 20
# speedup vs baseline: 1.1468x; 1.1468x over previous
"""MixedScoreMultiHeadAttention Trainium2 kernel (v3: flat-pipelined evac).

Data-parallel over batch: 32 batches -> 8 cores x 4 batches.

Per core (4 batches):
  setup: batched input DMAs (x per-t so transposes start early), PE
         transposes, q/k projections, per-head dots -> x4all [r,(h,b,c)]
         fp16; the SBUF->SBUF gather DMAs for rhs[b] = [17, R*C] are
         interleaved with the dot evacs so main(b0) starts early.
  main: ONE flat stream over 128 global chunks (8 iterations of (b, half)
        x 16 chunks of 1024 pts):
    gck:      layer1 matmul pair -> ps_ev slot (3 PSUM bufs break the
              evac->L1 WAR chain), relu+bias evac alternating ACT/DVE
    gck-2:    layer2 matmuls of the chunk two back (possibly previous
              (b,half)) so the PE never head-blocks the next iteration
    grp ends: exp evac [128,512] ACT -> wsb
    gck%16==4: AV + reciprocal + broadcast-normalize of the PREVIOUS
              (b,half), placed where their deps are already satisfied.

The relu evacuation of H*MS*R*C*BL values (131072 partition-columns) through
the only two PSUM-capable engines (ACT ~0.99 col/ns, DVE ~0.86 col/ns in
1024-col chunks) is the fundamental floor (~71us); everything else is
arranged to keep those two engines saturated.

mix2 bias b2 is dropped (softmax-invariant); 1/sqrt(D) folded into Wq.
"""
import sys

sys.path.insert(0, "/opt/trn_rl_repo")

import numpy as np
from contextlib import ExitStack

import concourse.bass as bass
import concourse.mybir as mybir
import concourse.tile as tile
from concourse import bacc
from concourse.bass_utils import run_bass_kernel_spmd
from concourse.masks import make_identity

B, R, C, E, H, D, MS = 32, 128, 128, 256, 16, 16, 16
NCORES = 8
BL = B // NCORES  # batches per core: 4
TOK = BL * R      # 512 tokens per core per side
PTS = R * C       # 16384 score points per (b)

FP32 = mybir.dt.float32
FP16 = mybir.dt.float16
AF = mybir.ActivationFunctionType
ALU = mybir.AluOpType


def build_kernel():
    nc = bacc.Bacc("TRN2", target_bir_lowering=False, debug=False,
                   num_devices=NCORES)

    x_r = nc.dram_tensor("x_r", [TOK, E], FP32, kind="ExternalInput").ap()
    x_c = nc.dram_tensor("x_c", [TOK, E], FP32, kind="ExternalInput").ap()
    cost = nc.dram_tensor("cost", [BL, R, C], FP32, kind="ExternalInput").ap()
    wq_d = nc.dram_tensor("Wq", [E, E], FP32, kind="ExternalInput").ap()
    wk_d = nc.dram_tensor("Wk", [E, E], FP32, kind="ExternalInput").ap()
    wv_d = nc.dram_tensor("Wv", [E, E], FP32, kind="ExternalInput").ap()
    # layer1 stationary [17, 256]: col (half*128 + (h%8)*16 + m):
    #   row h' = a[h,m] iff h'==h; row 16 = c[h,m]
    w1_d = nc.dram_tensor("W1L", [17, 2 * 128], FP32,
                          kind="ExternalInput").ap()
    # layer2 moving [128, 16]: col (half*8 + j): row hm = w2[half*8+j, m]
    # iff hm == ((j)*16+m) else 0
    w2_d = nc.dram_tensor("W2L", [128, 16], FP32, kind="ExternalInput").ap()
    # relu bias per (h,m) row: bcol2[hm, half] = b1[half*8 + hm//16, hm%16]
    bc_d = nc.dram_tensor("bcol2", [128, 2], FP32, kind="ExternalInput").ap()
    out_d = nc.dram_tensor("out", [BL, R, H * D], FP32,
                           kind="ExternalOutput").ap()

    with tile.TileContext(nc) as tc, ExitStack() as ctx:
        const_p = ctx.enter_context(tc.tile_pool(name="const", bufs=1))
        stage_p = ctx.enter_context(tc.tile_pool(name="stage", bufs=1))
        xt_p = ctx.enter_context(tc.tile_pool(name="xt", bufs=1))
        w_p = ctx.enter_context(tc.tile_pool(name="wts", bufs=1))
        quad_p = ctx.enter_context(tc.tile_pool(name="quad", bufs=1))
        x4_p = ctx.enter_context(tc.tile_pool(name="x4", bufs=1))
        rhs_p = ctx.enter_context(tc.tile_pool(name="rhs", bufs=1))
        rr_p = ctx.enter_context(tc.tile_pool(name="rr", bufs=3))
        wsb_p = ctx.enter_context(tc.tile_pool(name="wsb", bufs=1))
        fout_p = ctx.enter_context(tc.tile_pool(name="fout", bufs=1))
        small_p = ctx.enter_context(tc.tile_pool(name="small", bufs=2))
        # PSUM (8 banks): ev 3x[128,1024] = 6 banks, l2 1x[128,512] = 1,
        # av 1x[128,144] = 1
        ps_ev = ctx.enter_context(
            tc.tile_pool(name="psev", bufs=3, space="PSUM"))
        ps_l2 = ctx.enter_context(
            tc.tile_pool(name="psl2", bufs=1, space="PSUM"))
        ps_av = ctx.enter_context(
            tc.tile_pool(name="psav", bufs=1, space="PSUM"))

        # round-robin assignment of setup evac work to the two PSUM engines
        eng_i = [0]

        def evac_engine():
            eng_i[0] ^= 1
            return nc.scalar if eng_i[0] else nc.vector

        def evac_copy(dst, src):
            ev = evac_engine()
            (ev.copy if ev is nc.scalar else ev.tensor_copy)(dst, src)

        # ---- input DMAs (SP queue) ----
        w1f = stage_p.tile([17, 256], FP32, tag="w1f", name="w1f")
        nc.sync.dma_start(w1f[:], w1_d[:])
        w2f = stage_p.tile([128, 16], FP32, tag="w2f", name="w2f")
        nc.sync.dma_start(w2f[:], w2_d[:])
        bcol2 = const_p.tile([128, 2], FP32)
        nc.sync.dma_start(bcol2[:], bc_d[:])

        wdram = {"q": wq_d, "k": wk_d, "v": wv_d}

        def load_w32(name):
            w = stage_p.tile([128, 2, E], FP32, tag="w32", bufs=2,
                             name=f"w32{name}")
            nc.sync.dma_start(
                w[:], wdram[name][:].rearrange("(eh p) e -> p eh e", p=128))
            return w


        cost32 = stage_p.tile([128, BL, C], FP32, tag="cost32", name="cost32")
        nc.sync.dma_start(cost32[:], cost[:].rearrange("b r c -> r b c"))

        # ---- const/weight prep ----
        ident = const_p.tile([128, 128], FP32)
        make_identity(nc, ident[:])

        w1l = const_p.tile([17, 256], FP16)
        nc.vector.tensor_copy(w1l[:], w1f[:])
        w2l = const_p.tile([128, 16], FP16)
        nc.vector.tensor_copy(w2l[:], w2f[:])

        # q/k weights padded on-chip: head h -> 32-col slot; v unpadded
        wt16 = {}
        for name in ("q", "k"):
            w32 = load_w32(name)
            wt = w_p.tile([128, 2, 2 * E], FP16, tag=f"wt{name}",
                          name=f"wt{name}")
            nc.gpsimd.memset(wt[:], 0.0)
            wt4 = wt[:].rearrange("p eh (h x) -> p eh h x", h=H)
            w4 = w32[:].rearrange("p eh (h d) -> p eh h d", h=H)
            evac_copy(wt4[:, :, :, 0:D], w4[:])
            wt16[name] = wt
        w32v = load_w32("v")
        wtv = w_p.tile([128, 2, E], FP16, tag="wtv", name="wtv")
        nc.vector.tensor_copy(wtv[:], w32v[:])

        y16all = const_p.tile([128, BL * C], FP16, name="y16all")
        nc.scalar.copy(y16all[:], cost32[:].rearrange("p b c -> p (b c)"))

        # ---- x loads (rotating slices) + PE transposes ----
        xT = {}
        for name, dram in (("r", x_r), ("c", x_c)):
            xT[name] = [xt_p.tile([128, TOK], FP16, tag=f"xT{name}{eh}",
                                  name=f"xT{name}{eh}") for eh in range(2)]
            xsl = []
            for t in range(BL):
                xs = stage_p.tile([128, E], FP32, tag="xs", bufs=4,
                                  name="xs")
                nc.sync.dma_start(xs[:], dram[t * 128:(t + 1) * 128, :])
                xsl.append(xs)
            for eh in range(2):
                ps = ps_ev.tile([128, 1024], FP32, tag="ev", name="psev")
                for t in range(BL):
                    nc.tensor.transpose(
                        ps[:, t * 128:(t + 1) * 128],
                        xsl[t][:, eh * 128:(eh + 1) * 128], ident[:])
                evac_copy(xT[name][eh][:], ps[:, 0:512])

        # ---- projections + dots + gather, interleaved per quad-pair ----
        # (PE matmul operands must sit at base partition 0/32/64; 96 is
        #  invalid, so the 4th head of each quad lives in a base-0 tile)
        x4all = x4_p.tile([128, H * BL * C], FP16, name="x4all")
        x4v = x4all[:].rearrange("p (h b c) -> p h b c", h=H, b=BL)
        # single gathered rhs, b-major pts: [17, (b, r, c)] fp16.
        # One DMA per (head, b) keeps each DMA cheap in the tile scheduler's
        # internal (byte-cost-dominated) DMA model so later batches'
        # gathers aren't chained too deep behind main-loop progress.
        rhs_all = rhs_p.tile([17, BL * PTS], FP16, tag="rhs", name="rhs")

        def gather_h(h, b):
            nc.sync.dma_start(
                rhs_all[h:h + 1, b * PTS:(b + 1) * PTS], x4v[:, h, b, :])

        def proj_pair(proj, mhp, dst):
            ps = ps_ev.tile([128, 2 * TOK], FP32, tag="ev", name="psev")
            for i in range(2):
                mh = mhp * 2 + i
                for eh in range(2):
                    nc.tensor.matmul(
                        ps[:, i * TOK:(i + 1) * TOK],
                        wt16[proj][:, eh, mh * 128:(mh + 1) * 128],
                        xT["r" if proj == "q" else "c"][eh][:],
                        start=(eh == 0), stop=(eh == 1))
            quad = quad_p.tile([96, 2 * TOK], FP16, tag=f"{proj}Q{mhp}",
                               name=f"{proj}Q{mhp}")
            # both mhp's 4th heads share one tile: mhp0 rows 0:16,
            # mhp1 rows 32:48 (evac engines may shift partitions)
            lastp = quad_p.tile([48, 2 * TOK], FP16, tag=f"{proj}L",
                                bufs=1, name=f"{proj}L")
            evac_copy(quad[:], ps[0:96, :])
            evac_copy(lastp[mhp * 32:mhp * 32 + 16, :], ps[96:112, :])
            for i in range(2):
                for hh in range(4):
                    if hh < 3:
                        dst.append(quad[hh * 32:hh * 32 + 16,
                                        i * TOK:(i + 1) * TOK])
                    else:
                        dst.append(lastp[mhp * 32:mhp * 32 + 16,
                                         i * TOK:(i + 1) * TOK])

        qT, kT = [], []
        for mhp in range(2):
            proj_pair("q", mhp, qT)
            proj_pair("k", mhp, kT)
            for hp in range(4 * mhp, 4 * mhp + 4):  # head pairs
                ps = ps_ev.tile([128, 1024], FP32, tag="ev", name="psev")
                for i in range(2):
                    h = hp * 2 + i
                    for b in range(BL):
                        nc.tensor.matmul(
                            ps[:, i * 512 + b * 128:i * 512 + (b + 1) * 128],
                            qT[h][:, b * 128:(b + 1) * 128],
                            kT[h][:, b * 128:(b + 1) * 128])
                evac_copy(x4all[:, hp * 1024:(hp + 1) * 1024], ps[:])
                gather_h(hp * 2, 0)
                gather_h(hp * 2 + 1, 0)
        # cost row b0, then remaining batches (overlap the main loop)
        nc.sync.dma_start(rhs_all[16:17, 0:PTS], y16all[:, 0:128])
        for b in range(1, BL):
            for h in range(H):
                gather_h(h, b)
            nc.sync.dma_start(
                rhs_all[16:17, b * PTS:(b + 1) * PTS],
                y16all[:, b * 128:(b + 1) * 128])

        # ---- v projections (first needed at AV(b0), well into main) ----
        vhat = []
        for b in range(BL):
            vh = quad_p.tile([128, 17 * H], FP32, tag=f"vhat{b}",
                             name=f"vhat{b}")
            vh3 = vh[:].rearrange("p (h x) -> p h x", h=H)
            nc.gpsimd.memset(vh3[:, :, 16:17], 1.0)
            ps = ps_ev.tile([128, 1024], FP32, tag="ev", name="psev")
            for eh in range(2):
                nc.tensor.matmul(
                    ps[:, 0:E], xT["c"][eh][:, b * 128:(b + 1) * 128],
                    wtv[:, eh, :], start=(eh == 0), stop=(eh == 1))
            evac_copy(vh3[:, :, 0:16],
                      ps[:, 0:E].rearrange("p (h d) -> p h d", h=H))
            vhat.append(vh)

        # ---- main loop: flat stream over 128 global chunks ----
        fouts = [fout_p.tile([128, H * D], FP32, tag=f"fo{b}", name=f"fo{b}")
                 for b in range(BL)]

        NIT = 2 * BL          # 8 (b, half) iterations
        NCK = 16              # chunks per iteration
        LAG = 2               # layer2 lag in chunks
        state = {}            # per-iteration: ps2, wsb, rr chunks

        def emit_av(it):
            b, half = it // 2, it % 2
            wsb = state[it]["wsb"]
            psa_t = ps_av.tile([128, 144], FP32, tag="av", name="psa_t")
            psa = psa_t[:, 0:17 * 8]
            wsb3 = wsb[:].rearrange("p (s h) -> p s h", h=8)
            for hl in range(8):
                h = half * 8 + hl
                nc.tensor.matmul(
                    psa[:, hl * 17:(hl + 1) * 17],
                    wsb3[:, :, hl],
                    vhat[b][:, h * 17:(h + 1) * 17])
            psa3 = psa.rearrange("p (x y) -> p x y", x=8)
            rec = small_p.tile([128, 8], FP32, tag="rec", name="rec")
            nc.vector.reciprocal(rec[:], psa3[:, :, 16])
            fo3 = fouts[b][:, half * 128:(half + 1) * 128].rearrange(
                "p (x y) -> p x y", x=8)
            nc.vector.tensor_tensor(
                fo3[:], psa3[:, :, 0:16],
                rec[:].broadcast_to((128, 8, 16)), ALU.mult)
            if half == 1:
                nc.sync.dma_start(out_d[b], fouts[b][:])

        def emit_l1(gck):
            it, ck = gck // NCK, gck % NCK
            b, half = it // 2, it % 2
            if ck == 0:
                state[it] = {
                    "wsb": wsb_p.tile([128, 1024], FP32, tag="wsb",
                                      name="wsb"),
                    "ps2": [None, None],
                    "rr": {},
                }
            ps = ps_ev.tile([128, 1024], FP32, tag="ev", name="psev")
            base = b * PTS + ck * 1024
            for j in range(2):
                nc.tensor.matmul(
                    ps[:, j * 512:(j + 1) * 512],
                    w1l[:, half * 128:(half + 1) * 128],
                    rhs_all[:, base + j * 512:base + (j + 1) * 512])
            rr = rr_p.tile([128, 1024], FP16, tag="rr", name="rr")
            if ck % 2 == 0:
                nc.scalar.activation(rr[:], ps[:], AF.Relu,
                                     bias=bcol2[:, half:half + 1])
            else:
                nc.vector.tensor_scalar(rr[:], ps[:],
                                        bcol2[:, half:half + 1],
                                        0.0, ALU.add, ALU.max)
            state[it]["rr"][ck] = rr

        def emit_l2(gck):
            it, ck = gck // NCK, gck % NCK
            half = it % 2
            st = state[it]
            grp = ck // 8
            if ck % 8 == 0:
                st["ps2"][grp] = ps_l2.tile([128, 512], FP32, tag="l2",
                                           name="ps2")
            ps2 = st["ps2"][grp]
            rr = st["rr"].pop(ck)
            for s in range(8):
                rloc = (ck % 8) * 8 + s
                nc.tensor.matmul(
                    ps2[:, rloc * 8:rloc * 8 + 8],
                    rr[:, s * 128:(s + 1) * 128],
                    w2l[:, half * 8:(half + 1) * 8])
            if ck % 8 == 7:
                nc.scalar.activation(
                    st["wsb"][:, grp * 512:(grp + 1) * 512],
                    ps2[:], AF.Exp)

        TOTAL = NIT * NCK
        for gck in range(TOTAL + LAG):
            if gck < TOTAL:
                emit_l1(gck)
            if gck >= LAG:
                emit_l2(gck - LAG)
            if gck % NCK == 4 and gck // NCK >= 1 and gck < TOTAL:
                emit_av(gck // NCK - 1)
        emit_av(NIT - 1)

    nc.compile()
    return nc


_cache = {}


def kernel(**inputs):
    row_emb = np.asarray(inputs["row_emb"], dtype=np.float32)
    col_emb = np.asarray(inputs["col_emb"], dtype=np.float32)
    cost_mat = np.asarray(inputs["cost_mat"], dtype=np.float32)
    Wq = np.asarray(inputs["Wq"], dtype=np.float32)
    Wk = np.asarray(inputs["Wk"], dtype=np.float32)
    Wv = np.asarray(inputs["Wv"], dtype=np.float32)
    m1w = np.asarray(inputs["mix1_weight"], dtype=np.float32)
    m1b = np.asarray(inputs["mix1_bias"], dtype=np.float32)
    m2w = np.asarray(inputs["mix2_weight"], dtype=np.float32)

    a1 = m1w[:, 0, :]
    c1 = m1w[:, 1, :]
    w2 = m2w[:, :, 0]

    if "nc" not in _cache:
        _cache["nc"] = build_kernel()
    nc = _cache["nc"]

    wq_s = Wq * (1.0 / np.sqrt(D))

    w1l = np.zeros((17, 256), dtype=np.float32)
    w2l = np.zeros((128, 16), dtype=np.float32)
    bcol2 = np.zeros((128, 2), dtype=np.float32)
    for h in range(H):
        half, hl = h // 8, h % 8
        for m in range(MS):
            col = half * 128 + hl * 16 + m
            w1l[h, col] = a1[h, m]
            w1l[16, col] = c1[h, m]
            w2l[hl * 16 + m, half * 8 + hl] = w2[h, m]
            bcol2[hl * 16 + m, half] = m1b[h, m]

    in_maps = []
    for i in range(NCORES):
        sl = slice(i * BL, (i + 1) * BL)
        in_maps.append({
            "x_r": row_emb[sl].reshape(TOK, E),
            "x_c": col_emb[sl].reshape(TOK, E),
            "cost": cost_mat[sl],
            "Wq": wq_s, "Wk": Wk, "Wv": Wv,
            "W1L": w1l, "W2L": w2l, "bcol2": bcol2,
        })
    res = run_bass_kernel_spmd(nc, in_maps, list(range(NCORES)))
    out = np.concatenate([res.results[i]["out"] for i in range(NCORES)],
                         axis=0)
    return out.astype(np.float32)


# revision 23
# speedup vs baseline: 1.1724x; 1.0223x over previous
"""MixedScoreMultiHeadAttention Trainium2 kernel (v3: flat-pipelined evac).

Data-parallel over batch: 32 batches -> 8 cores x 4 batches.

Per core (4 batches):
  setup: batched input DMAs (x per-t so transposes start early), PE
         transposes, q/k projections, per-head dots -> x4all [r,(h,b,c)]
         fp16; the SBUF->SBUF gather DMAs for rhs[b] = [17, R*C] are
         interleaved with the dot evacs so main(b0) starts early.
  main: ONE flat stream over 128 global chunks (8 iterations of (b, half)
        x 16 chunks of 1024 pts):
    gck:      layer1 matmul pair -> ps_ev slot (3 PSUM bufs break the
              evac->L1 WAR chain), relu+bias evac alternating ACT/DVE
    gck-2:    layer2 matmuls of the chunk two back (possibly previous
              (b,half)) so the PE never head-blocks the next iteration
    grp ends: exp evac [128,512] ACT -> wsb
    gck%16==4: AV + reciprocal + broadcast-normalize of the PREVIOUS
              (b,half), placed where their deps are already satisfied.

The relu evacuation of H*MS*R*C*BL values (131072 partition-columns) through
the only two PSUM-capable engines (ACT ~0.99 col/ns, DVE ~0.86 col/ns in
1024-col chunks) is the fundamental floor (~71us); everything else is
arranged to keep those two engines saturated.

mix2 bias b2 is dropped (softmax-invariant); 1/sqrt(D) folded into Wq.
"""
import sys

sys.path.insert(0, "/opt/trn_rl_repo")

import numpy as np
from contextlib import ExitStack

import concourse.bass as bass
import concourse.mybir as mybir
import concourse.tile as tile
from concourse import bacc
from concourse.bass_utils import run_bass_kernel_spmd
from concourse.masks import make_identity

B, R, C, E, H, D, MS = 32, 128, 128, 256, 16, 16, 16
NCORES = 8
BL = B // NCORES  # batches per core: 4
TOK = BL * R      # 512 tokens per core per side
PTS = R * C       # 16384 score points per (b)

FP32 = mybir.dt.float32
FP16 = mybir.dt.float16
AF = mybir.ActivationFunctionType
ALU = mybir.AluOpType


def build_kernel():
    nc = bacc.Bacc("TRN2", target_bir_lowering=False, debug=False,
                   num_devices=NCORES)

    x_r = nc.dram_tensor("x_r", [TOK, E], FP32, kind="ExternalInput").ap()
    x_c = nc.dram_tensor("x_c", [TOK, E], FP32, kind="ExternalInput").ap()
    cost = nc.dram_tensor("cost", [BL, R, C], FP32, kind="ExternalInput").ap()
    wq_d = nc.dram_tensor("Wq", [E, E], FP32, kind="ExternalInput").ap()
    wk_d = nc.dram_tensor("Wk", [E, E], FP32, kind="ExternalInput").ap()
    wv_d = nc.dram_tensor("Wv", [E, E], FP32, kind="ExternalInput").ap()
    # layer1 stationary [17, 256]: col (half*128 + (h%8)*16 + m):
    #   row h' = a[h,m] iff h'==h; row 16 = c[h,m]
    w1_d = nc.dram_tensor("W1L", [17, 2 * 128], FP32,
                          kind="ExternalInput").ap()
    # layer2 moving [128, 16]: col (half*8 + j): row hm = w2[half*8+j, m]
    # iff hm == ((j)*16+m) else 0
    w2_d = nc.dram_tensor("W2L", [128, 16], FP32, kind="ExternalInput").ap()
    # relu bias per (h,m) row: bcol2[hm, half] = b1[half*8 + hm//16, hm%16]
    bc_d = nc.dram_tensor("bcol2", [128, 2], FP32, kind="ExternalInput").ap()
    out_d = nc.dram_tensor("out", [BL, R, H * D], FP32,
                           kind="ExternalOutput").ap()

    with tile.TileContext(nc) as tc, ExitStack() as ctx:
        const_p = ctx.enter_context(tc.tile_pool(name="const", bufs=1))
        stage_p = ctx.enter_context(tc.tile_pool(name="stage", bufs=1))
        xt_p = ctx.enter_context(tc.tile_pool(name="xt", bufs=1))
        w_p = ctx.enter_context(tc.tile_pool(name="wts", bufs=1))
        quad_p = ctx.enter_context(tc.tile_pool(name="quad", bufs=1))
        x4_p = ctx.enter_context(tc.tile_pool(name="x4", bufs=1))
        rhs_p = ctx.enter_context(tc.tile_pool(name="rhs", bufs=1))
        rr_p = ctx.enter_context(tc.tile_pool(name="rr", bufs=3))
        wsb_p = ctx.enter_context(tc.tile_pool(name="wsb", bufs=1))
        fout_p = ctx.enter_context(tc.tile_pool(name="fout", bufs=1))
        small_p = ctx.enter_context(tc.tile_pool(name="small", bufs=2))
        # PSUM (8 banks): ev 3x[128,1024] = 6 banks, l2 1x[128,512] = 1,
        # av 1x[128,144] = 1
        ps_ev = ctx.enter_context(
            tc.tile_pool(name="psev", bufs=3, space="PSUM"))
        ps_l2 = ctx.enter_context(
            tc.tile_pool(name="psl2", bufs=1, space="PSUM"))
        ps_av = ctx.enter_context(
            tc.tile_pool(name="psav", bufs=1, space="PSUM"))

        # round-robin assignment of setup evac work to the two PSUM engines
        eng_i = [0]

        def evac_engine():
            eng_i[0] ^= 1
            return nc.scalar if eng_i[0] else nc.vector

        def evac_copy(dst, src):
            ev = evac_engine()
            (ev.copy if ev is nc.scalar else ev.tensor_copy)(dst, src)

        # ---- input DMAs (SP queue) ----
        w1f = stage_p.tile([17, 256], FP32, tag="w1f", name="w1f")
        nc.sync.dma_start(w1f[:], w1_d[:])
        w2f = stage_p.tile([128, 16], FP32, tag="w2f", name="w2f")
        nc.sync.dma_start(w2f[:], w2_d[:])
        bcol2 = const_p.tile([128, 2], FP32)
        nc.sync.dma_start(bcol2[:], bc_d[:])

        wdram = {"q": wq_d, "k": wk_d, "v": wv_d}

        def load_w32(name):
            w = stage_p.tile([128, 2, E], FP32, tag="w32", bufs=2,
                             name=f"w32{name}")
            nc.sync.dma_start(
                w[:], wdram[name][:].rearrange("(eh p) e -> p eh e", p=128))
            return w

        # x slices first on the SP queue: the transpose chain gates setup
        xsl = {}
        for name, dram in (("r", x_r), ("c", x_c)):
            xsl[name] = []
            for t in range(BL):
                xs = stage_p.tile([128, E], FP32, tag="xs", bufs=8,
                                  name="xs")
                nc.sync.dma_start(xs[:], dram[t * 128:(t + 1) * 128, :])
                xsl[name].append(xs)

        cost32 = stage_p.tile([128, BL, C], FP32, tag="cost32", name="cost32")
        nc.sync.dma_start(cost32[:], cost[:].rearrange("b r c -> r b c"))

        # ---- const/weight prep ----
        ident = const_p.tile([128, 128], FP32)
        make_identity(nc, ident[:])

        w1l = const_p.tile([17, 256], FP16)
        nc.vector.tensor_copy(w1l[:], w1f[:])
        w2l = const_p.tile([128, 16], FP16)
        nc.vector.tensor_copy(w2l[:], w2f[:])

        # q/k weights padded on-chip: head h -> 32-col slot; v unpadded
        wt16 = {}
        for name in ("q", "k"):
            w32 = load_w32(name)
            wt = w_p.tile([128, 2, 2 * E], FP16, tag=f"wt{name}",
                          name=f"wt{name}")
            nc.gpsimd.memset(wt[:], 0.0)
            wt4 = wt[:].rearrange("p eh (h x) -> p eh h x", h=H)
            w4 = w32[:].rearrange("p eh (h d) -> p eh h d", h=H)
            evac_copy(wt4[:, :, :, 0:D], w4[:])
            wt16[name] = wt
        w32v = load_w32("v")
        wtv = w_p.tile([128, 2, E], FP16, tag="wtv", name="wtv")
        nc.vector.tensor_copy(wtv[:], w32v[:])

        y16all = const_p.tile([128, BL * C], FP16, name="y16all")
        nc.scalar.copy(y16all[:], cost32[:].rearrange("p b c -> p (b c)"))

        # ---- PE transposes of the x slices ----
        xT = {}
        for name in ("r", "c"):
            xT[name] = [xt_p.tile([128, TOK], FP16, tag=f"xT{name}{eh}",
                                  name=f"xT{name}{eh}") for eh in range(2)]
            for eh in range(2):
                ps = ps_ev.tile([128, 1024], FP32, tag="ev", name="psev")
                for t in range(BL):
                    nc.tensor.transpose(
                        ps[:, t * 128:(t + 1) * 128],
                        xsl[name][t][:, eh * 128:(eh + 1) * 128], ident[:])
                evac_copy(xT[name][eh][:], ps[:, 0:512])

        # ---- projections + dots + gather, interleaved per quad-pair ----
        # (PE matmul operands must sit at base partition 0/32/64; 96 is
        #  invalid, so the 4th head of each quad lives in a base-0 tile)
        x4all = x4_p.tile([128, H * BL * C], FP16, name="x4all")
        x4v = x4all[:].rearrange("p (h b c) -> p h b c", h=H, b=BL)
        # single gathered rhs, b-major pts: [17, (b, r, c)] fp16.
        # One DMA per (head, b) keeps each DMA cheap in the tile scheduler's
        # internal (byte-cost-dominated) DMA model so later batches'
        # gathers aren't chained too deep behind main-loop progress.
        rhs_all = rhs_p.tile([17, BL * PTS], FP16, tag="rhs", name="rhs")

        def gather_h(h, b):
            nc.sync.dma_start(
                rhs_all[h:h + 1, b * PTS:(b + 1) * PTS], x4v[:, h, b, :])

        # head-GROUPS of 3 (heads at psum rows 0/32/64 -- all valid matmul
        # base partitions, unlike 96): 6 groups of (3,3,3,3,3,1) heads
        GSZ = [3, 3, 3, 3, 3, 1]
        GOF = [0, 3, 6, 9, 12, 15]
        qAll = {"q": quad_p.tile([96, 6 * TOK], FP16, tag="qAll",
                                 name="qAll"),
                "k": quad_p.tile([96, 6 * TOK], FP16, tag="kAll",
                                 name="kAll")}

        def proj_groups(proj, gp, dst):
            # two 3-head groups share one [128,1024] psum slot
            ps = ps_ev.tile([128, 2 * TOK], FP32, tag="ev", name="psev")
            for i in range(2):
                g = gp * 2 + i
                for eh in range(2):
                    nc.tensor.matmul(
                        ps[0:32 * GSZ[g], i * TOK:(i + 1) * TOK],
                        wt16[proj][:, eh, GOF[g] * 32:
                                   (GOF[g] + GSZ[g]) * 32],
                        xT["r" if proj == "q" else "c"][eh][:],
                        start=(eh == 0), stop=(eh == 1))
            for i in range(2):
                g = gp * 2 + i
                evac_copy(qAll[proj][0:32 * GSZ[g],
                                     g * TOK:(g + 1) * TOK],
                          ps[0:32 * GSZ[g], i * TOK:(i + 1) * TOK])

        def head_slice(proj, h):
            g, loc = h // 3, h % 3
            return qAll[proj][loc * 32:loc * 32 + 16,
                              g * TOK:(g + 1) * TOK]

        qT = [head_slice("q", h) for h in range(H)]
        kT = [head_slice("k", h) for h in range(H)]
        for gp in range(3):
            proj_groups("q", gp, None)
            proj_groups("k", gp, None)
            for hp in range(3 * gp, min(3 * gp + 3, 8)):  # head pairs
                ps = ps_ev.tile([128, 1024], FP32, tag="ev", name="psev")
                for i in range(2):
                    h = hp * 2 + i
                    for b in range(BL):
                        nc.tensor.matmul(
                            ps[:, i * 512 + b * 128:i * 512 + (b + 1) * 128],
                            qT[h][:, b * 128:(b + 1) * 128],
                            kT[h][:, b * 128:(b + 1) * 128])
                evac_copy(x4all[:, hp * 1024:(hp + 1) * 1024], ps[:])
                gather_h(hp * 2, 0)
                gather_h(hp * 2 + 1, 0)
        # cost row b0, then remaining batches (overlap the main loop)
        nc.sync.dma_start(rhs_all[16:17, 0:PTS], y16all[:, 0:128])
        for b in range(1, BL):
            for h in range(H):
                gather_h(h, b)
            nc.sync.dma_start(
                rhs_all[16:17, b * PTS:(b + 1) * PTS],
                y16all[:, b * 128:(b + 1) * 128])

        # ---- v projections (first needed at AV(b0), well into main) ----
        vhat = []
        for b in range(BL):
            vh = quad_p.tile([128, 17 * H], FP32, tag=f"vhat{b}",
                             name=f"vhat{b}")
            vh3 = vh[:].rearrange("p (h x) -> p h x", h=H)
            nc.gpsimd.memset(vh3[:, :, 16:17], 1.0)
            ps = ps_ev.tile([128, 1024], FP32, tag="ev", name="psev")
            for eh in range(2):
                nc.tensor.matmul(
                    ps[:, 0:E], xT["c"][eh][:, b * 128:(b + 1) * 128],
                    wtv[:, eh, :], start=(eh == 0), stop=(eh == 1))
            evac_copy(vh3[:, :, 0:16],
                      ps[:, 0:E].rearrange("p (h d) -> p h d", h=H))
            vhat.append(vh)

        # ---- main loop: flat stream over 128 global chunks ----
        fouts = [fout_p.tile([128, H * D], FP32, tag=f"fo{b}", name=f"fo{b}")
                 for b in range(BL)]

        NIT = 2 * BL          # 8 (b, half) iterations
        NCK = 16              # chunks per iteration
        LAG = 2               # layer2 lag in chunks
        state = {}            # per-iteration: ps2, wsb, rr chunks

        def emit_av(it):
            b, half = it // 2, it % 2
            wsb = state[it]["wsb"]
            psa_t = ps_av.tile([128, 144], FP32, tag="av", name="psa_t")
            psa = psa_t[:, 0:17 * 8]
            wsb3 = wsb[:].rearrange("p (s h) -> p s h", h=8)
            for hl in range(8):
                h = half * 8 + hl
                nc.tensor.matmul(
                    psa[:, hl * 17:(hl + 1) * 17],
                    wsb3[:, :, hl],
                    vhat[b][:, h * 17:(h + 1) * 17])
            psa3 = psa.rearrange("p (x y) -> p x y", x=8)
            rec = small_p.tile([128, 8], FP32, tag="rec", name="rec")
            nc.vector.reciprocal(rec[:], psa3[:, :, 16])
            fo3 = fouts[b][:, half * 128:(half + 1) * 128].rearrange(
                "p (x y) -> p x y", x=8)
            nc.vector.tensor_tensor(
                fo3[:], psa3[:, :, 0:16],
                rec[:].broadcast_to((128, 8, 16)), ALU.mult)
            if half == 1:
                nc.sync.dma_start(out_d[b], fouts[b][:])

        def emit_l1(gck):
            it, ck = gck // NCK, gck % NCK
            b, half = it // 2, it % 2
            if ck == 0:
                state[it] = {
                    "wsb": wsb_p.tile([128, 1024], FP32, tag="wsb",
                                      name="wsb"),
                    "ps2": [None, None],
                    "rr": {},
                }
            ps = ps_ev.tile([128, 1024], FP32, tag="ev", name="psev")
            base = b * PTS + ck * 1024
            for j in range(2):
                nc.tensor.matmul(
                    ps[:, j * 512:(j + 1) * 512],
                    w1l[:, half * 128:(half + 1) * 128],
                    rhs_all[:, base + j * 512:base + (j + 1) * 512])
            rr = rr_p.tile([128, 1024], FP16, tag="rr", name="rr")
            to_act = (ck % 2 == 0) or (ck == 13 and it in (2, 5))
            if to_act:
                nc.scalar.activation(rr[:], ps[:], AF.Relu,
                                     bias=bcol2[:, half:half + 1])
            else:
                nc.vector.tensor_scalar(rr[:], ps[:],
                                        bcol2[:, half:half + 1],
                                        0.0, ALU.add, ALU.max)
            state[it]["rr"][ck] = rr

        def emit_l2(gck):
            it, ck = gck // NCK, gck % NCK
            half = it % 2
            st = state[it]
            grp = ck // 8
            if ck % 8 == 0:
                st["ps2"][grp] = ps_l2.tile([128, 512], FP32, tag="l2",
                                           name="ps2")
            ps2 = st["ps2"][grp]
            rr = st["rr"].pop(ck)
            for s in range(8):
                rloc = (ck % 8) * 8 + s
                nc.tensor.matmul(
                    ps2[:, rloc * 8:rloc * 8 + 8],
                    rr[:, s * 128:(s + 1) * 128],
                    w2l[:, half * 8:(half + 1) * 8])
            if ck % 8 == 7:
                nc.scalar.activation(
                    st["wsb"][:, grp * 512:(grp + 1) * 512],
                    ps2[:], AF.Exp)

        TOTAL = NIT * NCK
        for gck in range(TOTAL + LAG):
            if gck < TOTAL:
                emit_l1(gck)
            if gck >= LAG:
                emit_l2(gck - LAG)
            if gck % NCK == 4 and gck // NCK >= 1 and gck < TOTAL:
                emit_av(gck // NCK - 1)
        emit_av(NIT - 1)

    nc.compile()
    return nc


_cache = {}


def kernel(**inputs):
    row_emb = np.asarray(inputs["row_emb"], dtype=np.float32)
    col_emb = np.asarray(inputs["col_emb"], dtype=np.float32)
    cost_mat = np.asarray(inputs["cost_mat"], dtype=np.float32)
    Wq = np.asarray(inputs["Wq"], dtype=np.float32)
    Wk = np.asarray(inputs["Wk"], dtype=np.float32)
    Wv = np.asarray(inputs["Wv"], dtype=np.float32)
    m1w = np.asarray(inputs["mix1_weight"], dtype=np.float32)
    m1b = np.asarray(inputs["mix1_bias"], dtype=np.float32)
    m2w = np.asarray(inputs["mix2_weight"], dtype=np.float32)

    a1 = m1w[:, 0, :]
    c1 = m1w[:, 1, :]
    w2 = m2w[:, :, 0]

    if "nc" not in _cache:
        _cache["nc"] = build_kernel()
    nc = _cache["nc"]

    wq_s = Wq * (1.0 / np.sqrt(D))

    w1l = np.zeros((17, 256), dtype=np.float32)
    w2l = np.zeros((128, 16), dtype=np.float32)
    bcol2 = np.zeros((128, 2), dtype=np.float32)
    for h in range(H):
        half, hl = h // 8, h % 8
        for m in range(MS):
            col = half * 128 + hl * 16 + m
            w1l[h, col] = a1[h, m]
            w1l[16, col] = c1[h, m]
            w2l[hl * 16 + m, half * 8 + hl] = w2[h, m]
            bcol2[hl * 16 + m, half] = m1b[h, m]

    in_maps = []
    for i in range(NCORES):
        sl = slice(i * BL, (i + 1) * BL)
        in_maps.append({
            "x_r": row_emb[sl].reshape(TOK, E),
            "x_c": col_emb[sl].reshape(TOK, E),
            "cost": cost_mat[sl],
            "Wq": wq_s, "Wk": Wk, "Wv": Wv,
            "W1L": w1l, "W2L": w2l, "bcol2": bcol2,
        })
    res = run_bass_kernel_spmd(nc, in_maps, list(range(NCORES)))
    out = np.concatenate([res.results[i]["out"] for i in range(NCORES)],
                         axis=0)
    return out.astype(np.float32)


# revision 33
# speedup vs baseline: 1.2107x; 1.0327x over previous
"""MixedScoreMultiHeadAttention Trainium2 kernel (v3: flat-pipelined evac).

Data-parallel over batch: 32 batches -> 8 cores x 4 batches.

Per core (4 batches):
  setup: batched input DMAs (x per-t so transposes start early), PE
         transposes, q/k projections, per-head dots -> x4all [r,(h,b,c)]
         fp16; the SBUF->SBUF gather DMAs for rhs[b] = [17, R*C] are
         interleaved with the dot evacs so main(b0) starts early.
  main: ONE flat stream over 128 global chunks (8 iterations of (b, half)
        x 16 chunks of 1024 pts):
    gck:      layer1 matmul pair -> ps_ev slot (3 PSUM bufs break the
              evac->L1 WAR chain), relu+bias evac alternating ACT/DVE
    gck-2:    layer2 matmuls of the chunk two back (possibly previous
              (b,half)) so the PE never head-blocks the next iteration
    grp ends: exp evac [128,512] ACT -> wsb
    gck%16==4: AV + reciprocal + broadcast-normalize of the PREVIOUS
              (b,half), placed where their deps are already satisfied.

The relu evacuation of H*MS*R*C*BL values (131072 partition-columns) through
the only two PSUM-capable engines (ACT ~0.99 col/ns, DVE ~0.86 col/ns in
1024-col chunks) is the fundamental floor (~71us); everything else is
arranged to keep those two engines saturated.

mix2 bias b2 is dropped (softmax-invariant); 1/sqrt(D) folded into Wq.
"""
import sys

sys.path.insert(0, "/opt/trn_rl_repo")

import numpy as np
from contextlib import ExitStack

import concourse.bass as bass
import concourse.mybir as mybir
import concourse.tile as tile
from concourse import bacc
from concourse.bass_utils import run_bass_kernel_spmd
from concourse.masks import make_identity

B, R, C, E, H, D, MS = 32, 128, 128, 256, 16, 16, 16
NCORES = 8
BL = B // NCORES  # batches per core: 4
TOK = BL * R      # 512 tokens per core per side
PTS = R * C       # 16384 score points per (b)

FP32 = mybir.dt.float32
FP16 = mybir.dt.float16
AF = mybir.ActivationFunctionType
ALU = mybir.AluOpType


def build_kernel():
    nc = bacc.Bacc("TRN2", target_bir_lowering=False, debug=False,
                   num_devices=NCORES)

    x_r = nc.dram_tensor("x_r", [TOK, E], FP32, kind="ExternalInput").ap()
    x_c = nc.dram_tensor("x_c", [TOK, E], FP32, kind="ExternalInput").ap()
    cost = nc.dram_tensor("cost", [BL, R, C], FP32, kind="ExternalInput").ap()
    wq_d = nc.dram_tensor("Wq", [E, E], FP32, kind="ExternalInput").ap()
    wk_d = nc.dram_tensor("Wk", [E, E], FP32, kind="ExternalInput").ap()
    wv_d = nc.dram_tensor("Wv", [E, E], FP32, kind="ExternalInput").ap()
    # layer1 stationary [17, 256]: col (half*128 + (h%8)*16 + m):
    #   row h' = a[h,m] iff h'==h; row 16 = c[h,m]
    w1_d = nc.dram_tensor("W1L", [17, 2 * 128], FP32,
                          kind="ExternalInput").ap()
    # layer2 moving [128, 16]: col (half*8 + j): row hm = w2[half*8+j, m]
    # iff hm == ((j)*16+m) else 0
    w2_d = nc.dram_tensor("W2L", [128, 16], FP32, kind="ExternalInput").ap()
    # relu bias per (h,m) row: bcol2[hm, half] = b1[half*8 + hm//16, hm%16]
    bc_d = nc.dram_tensor("bcol2", [128, 2], FP32, kind="ExternalInput").ap()
    out_d = nc.dram_tensor("out", [BL, R, H * D], FP32,
                           kind="ExternalOutput").ap()

    with tile.TileContext(nc) as tc, ExitStack() as ctx:
        const_p = ctx.enter_context(tc.tile_pool(name="const", bufs=1))
        stage_p = ctx.enter_context(tc.tile_pool(name="stage", bufs=1))
        xt_p = ctx.enter_context(tc.tile_pool(name="xt", bufs=1))
        w_p = ctx.enter_context(tc.tile_pool(name="wts", bufs=1))
        quad_p = ctx.enter_context(tc.tile_pool(name="quad", bufs=1))
        x4_p = ctx.enter_context(tc.tile_pool(name="x4", bufs=1))
        rhs_p = ctx.enter_context(tc.tile_pool(name="rhs", bufs=1))
        rr_p = ctx.enter_context(tc.tile_pool(name="rr", bufs=4))
        wsb_p = ctx.enter_context(tc.tile_pool(name="wsb", bufs=1))
        fout_p = ctx.enter_context(tc.tile_pool(name="fout", bufs=1))
        small_p = ctx.enter_context(tc.tile_pool(name="small", bufs=2))
        # PSUM (8 banks): ev 3x[128,1024] = 6 banks, l2 1x[128,512] = 1,
        # av 1x[128,144] = 1
        ps_ev = ctx.enter_context(
            tc.tile_pool(name="psev", bufs=3, space="PSUM"))
        ps_l2 = ctx.enter_context(
            tc.tile_pool(name="psl2", bufs=1, space="PSUM"))
        ps_av = ctx.enter_context(
            tc.tile_pool(name="psav", bufs=1, space="PSUM"))

        # round-robin assignment of setup evac work to the two PSUM engines
        eng_i = [0]

        def evac_engine():
            eng_i[0] ^= 1
            return nc.scalar if eng_i[0] else nc.vector

        def evac_copy(dst, src):
            ev = evac_engine()
            (ev.copy if ev is nc.scalar else ev.tensor_copy)(dst, src)

        # ---- input DMAs (SP queue) ----
        w1f = stage_p.tile([17, 256], FP32, tag="w1f", name="w1f")
        nc.sync.dma_start(w1f[:], w1_d[:])
        w2f = stage_p.tile([128, 16], FP32, tag="w2f", name="w2f")
        nc.sync.dma_start(w2f[:], w2_d[:])
        bcol2 = const_p.tile([128, 2], FP32)
        nc.sync.dma_start(bcol2[:], bc_d[:])

        wdram = {"q": wq_d, "k": wk_d, "v": wv_d}

        def load_w32(name):
            w = stage_p.tile([128, 2, E], FP32, tag="w32", bufs=2,
                             name=f"w32{name}")
            nc.sync.dma_start(
                w[:], wdram[name][:].rearrange("(eh p) e -> p eh e", p=128))
            return w

        # x first on the SP queue (the transpose chain gates setup); one
        # batched DMA per side -- HWDGE pays 625ns fixed per DMA
        x32 = {}
        for name, dram in (("r", x_r), ("c", x_c)):
            x32[name] = stage_p.tile([128, BL, E], FP32, tag=f"x32{name}",
                                     name=f"x32{name}")
            nc.sync.dma_start(
                x32[name][:], dram[:].rearrange("(t p) e -> p t e", p=128))
        xsl = {name: [x32[name][:, t, :] for t in range(BL)]
               for name in ("r", "c")}

        cost32 = stage_p.tile([128, BL, C], FP32, tag="cost32", name="cost32")
        nc.sync.dma_start(cost32[:], cost[:].rearrange("b r c -> r b c"))

        # ---- const/weight prep ----
        ident = const_p.tile([128, 128], FP32)
        make_identity(nc, ident[:])

        w1l = const_p.tile([17, 256], FP16)
        nc.gpsimd.tensor_copy(w1l[:], w1f[:])
        w2l = const_p.tile([128, 16], FP16)
        nc.gpsimd.tensor_copy(w2l[:], w2f[:])

        # q/k weights padded on-chip: head h -> 32-col slot; v unpadded
        wt16 = {}
        for name in ("q", "k"):
            w32 = load_w32(name)
            wt = w_p.tile([128, 2, 2 * E], FP16, tag=f"wt{name}",
                          name=f"wt{name}")
            nc.gpsimd.memset(wt[:], 0.0)
            wt4 = wt[:].rearrange("p eh (h x) -> p eh h x", h=H)
            w4 = w32[:].rearrange("p eh (h d) -> p eh h d", h=H)
            nc.gpsimd.tensor_copy(wt4[:, :, :, 0:D], w4[:])
            wt16[name] = wt
        w32v = load_w32("v")
        wtv = w_p.tile([128, 2, E], FP16, tag="wtv", name="wtv")
        nc.gpsimd.tensor_copy(wtv[:], w32v[:])

        y16all = const_p.tile([128, BL * C], FP16, name="y16all")
        nc.gpsimd.tensor_copy(y16all[:], cost32[:].rearrange("p b c -> p (b c)"))

        # ---- PE transposes of the x slices ----
        xT = {}
        for name in ("r", "c"):
            xT[name] = [xt_p.tile([128, TOK], FP16, tag=f"xT{name}{eh}",
                                  name=f"xT{name}{eh}") for eh in range(2)]
            for eh in range(2):
                ps = ps_ev.tile([128, 1024], FP32, tag="ev", name="psev")
                for t in range(BL):
                    nc.tensor.transpose(
                        ps[:, t * 128:(t + 1) * 128],
                        xsl[name][t][:, eh * 128:(eh + 1) * 128], ident[:])
                evac_copy(xT[name][eh][:], ps[:, 0:512])

        # ---- projections + dots + gather, interleaved per quad-pair ----
        # (PE matmul operands must sit at base partition 0/32/64; 96 is
        #  invalid, so the 4th head of each quad lives in a base-0 tile)
        x4all = x4_p.tile([128, H * BL * C], FP16, name="x4all")
        x4v = x4all[:].rearrange("p (h b c) -> p h b c", h=H, b=BL)
        # single gathered rhs, b-major pts: [17, (b, r, c)] fp16.
        # One DMA per (head, b) keeps each DMA cheap in the tile scheduler's
        # internal (byte-cost-dominated) DMA model so later batches'
        # gathers aren't chained too deep behind main-loop progress.
        rhs_all = rhs_p.tile([17, BL * PTS], FP16, tag="rhs", name="rhs")

        def gather_h(h, b):
            nc.sync.dma_start(
                rhs_all[h:h + 1, b * PTS:(b + 1) * PTS], x4v[:, h, b, :])

        # head-GROUPS of 3 (heads at psum rows 0/32/64 -- all valid matmul
        # base partitions, unlike 96): 6 groups of (3,3,3,3,3,1) heads
        GSZ = [3, 3, 3, 3, 3, 1]
        GOF = [0, 3, 6, 9, 12, 15]
        qAll = {"q": quad_p.tile([96, 6 * TOK], FP16, tag="qAll",
                                 name="qAll"),
                "k": quad_p.tile([96, 6 * TOK], FP16, tag="kAll",
                                 name="kAll")}

        def proj_groups(proj, gp, dst):
            # two 3-head groups share one [128,1024] psum slot
            ps = ps_ev.tile([128, 2 * TOK], FP32, tag="ev", name="psev")
            for i in range(2):
                g = gp * 2 + i
                for eh in range(2):
                    nc.tensor.matmul(
                        ps[0:32 * GSZ[g], i * TOK:(i + 1) * TOK],
                        wt16[proj][:, eh, GOF[g] * 32:
                                   (GOF[g] + GSZ[g]) * 32],
                        xT["r" if proj == "q" else "c"][eh][:],
                        start=(eh == 0), stop=(eh == 1))
            for i in range(2):
                g = gp * 2 + i
                evac_copy(qAll[proj][0:32 * GSZ[g],
                                     g * TOK:(g + 1) * TOK],
                          ps[0:32 * GSZ[g], i * TOK:(i + 1) * TOK])

        def head_slice(proj, h):
            g, loc = h // 3, h % 3
            return qAll[proj][loc * 32:loc * 32 + 16,
                              g * TOK:(g + 1) * TOK]

        qT = [head_slice("q", h) for h in range(H)]
        kT = [head_slice("k", h) for h in range(H)]
        for gp in range(3):
            proj_groups("q", gp, None)
            proj_groups("k", gp, None)
            for hp in range(3 * gp, min(3 * gp + 3, 8)):  # head pairs
                ps = ps_ev.tile([128, 1024], FP32, tag="ev", name="psev")
                for i in range(2):
                    h = hp * 2 + i
                    for b in range(BL):
                        nc.tensor.matmul(
                            ps[:, i * 512 + b * 128:i * 512 + (b + 1) * 128],
                            qT[h][:, b * 128:(b + 1) * 128],
                            kT[h][:, b * 128:(b + 1) * 128])
                evac_copy(x4all[:, hp * 1024:(hp + 1) * 1024], ps[:])
                gather_h(hp * 2, 0)
                gather_h(hp * 2 + 1, 0)
        # cost row b0, then remaining batches (overlap the main loop)
        nc.sync.dma_start(rhs_all[16:17, 0:PTS], y16all[:, 0:128])
        for b in range(1, BL):
            for h in range(H):
                gather_h(h, b)
            nc.sync.dma_start(
                rhs_all[16:17, b * PTS:(b + 1) * PTS],
                y16all[:, b * 128:(b + 1) * 128])

        # ---- v projections (first needed at AV(b0), well into main) ----
        vhat = []
        for b in range(BL):
            vh = quad_p.tile([128, 17 * H], FP32, tag=f"vhat{b}",
                             name=f"vhat{b}")
            vh3 = vh[:].rearrange("p (h x) -> p h x", h=H)
            nc.gpsimd.memset(vh3[:, :, 16:17], 1.0)
            ps = ps_ev.tile([128, 1024], FP32, tag="ev", name="psev")
            for eh in range(2):
                nc.tensor.matmul(
                    ps[:, 0:E], xT["c"][eh][:, b * 128:(b + 1) * 128],
                    wtv[:, eh, :], start=(eh == 0), stop=(eh == 1))
            evac_copy(vh3[:, :, 0:16],
                      ps[:, 0:E].rearrange("p (h d) -> p h d", h=H))
            vhat.append(vh)

        # ---- main loop: flat stream over 128 global chunks ----
        fouts = [fout_p.tile([128, H * D], FP32, tag=f"fo{b}", name=f"fo{b}")
                 for b in range(BL)]

        NIT = 2 * BL          # 8 (b, half) iterations
        NCK = 16              # chunks per iteration
        LAG = 2               # layer2 lag in chunks
        state = {}            # per-iteration: ps2, wsb, rr chunks

        def emit_av(it):
            b, half = it // 2, it % 2
            wsb = state[it]["wsb"]
            psa_t = ps_av.tile([128, 144], FP32, tag="av", name="psa_t")
            psa = psa_t[:, 0:17 * 8]
            wsb3 = wsb[:].rearrange("p (s h) -> p s h", h=8)
            for hl in range(8):
                h = half * 8 + hl
                nc.tensor.matmul(
                    psa[:, hl * 17:(hl + 1) * 17],
                    wsb3[:, :, hl],
                    vhat[b][:, h * 17:(h + 1) * 17])
            psa3 = psa.rearrange("p (x y) -> p x y", x=8)
            rec = small_p.tile([128, 8], FP32, tag="rec", name="rec")
            nc.vector.reciprocal(rec[:], psa3[:, :, 16])
            fo3 = fouts[b][:, half * 128:(half + 1) * 128].rearrange(
                "p (x y) -> p x y", x=8)
            nc.vector.tensor_tensor(
                fo3[:], psa3[:, :, 0:16],
                rec[:].broadcast_to((128, 8, 16)), ALU.mult)
            if half == 1:
                nc.sync.dma_start(out_d[b], fouts[b][:])

        def emit_l1(gck):
            it, ck = gck // NCK, gck % NCK
            b, half = it // 2, it % 2
            if ck == 0:
                state[it] = {
                    "wsb": wsb_p.tile([128, 1024], FP32, tag="wsb",
                                      name="wsb"),
                    "ps2": [None, None],
                    "rr": {},
                }
            ps = ps_ev.tile([128, 1024], FP32, tag="ev", name="psev")
            base = b * PTS + ck * 1024
            for j in range(2):
                nc.tensor.matmul(
                    ps[:, j * 512:(j + 1) * 512],
                    w1l[:, half * 128:(half + 1) * 128],
                    rhs_all[:, base + j * 512:base + (j + 1) * 512])
            rr = rr_p.tile([128, 1024], FP16, tag="rr", name="rr")
            to_act = ck % 2 == 0
            if to_act:
                nc.scalar.activation(rr[:], ps[:], AF.Relu,
                                     bias=bcol2[:, half:half + 1])
            else:
                nc.vector.tensor_scalar(rr[:], ps[:],
                                        bcol2[:, half:half + 1],
                                        0.0, ALU.add, ALU.max)
            state[it]["rr"][ck] = rr

        def emit_l2(gck):
            it, ck = gck // NCK, gck % NCK
            half = it % 2
            st = state[it]
            grp = ck // 8
            if ck % 8 == 0:
                st["ps2"][grp] = ps_l2.tile([128, 512], FP32, tag="l2",
                                           name="ps2")
            ps2 = st["ps2"][grp]
            rr = st["rr"].pop(ck)
            for s in range(8):
                rloc = (ck % 8) * 8 + s
                nc.tensor.matmul(
                    ps2[:, rloc * 8:rloc * 8 + 8],
                    rr[:, s * 128:(s + 1) * 128],
                    w2l[:, half * 8:(half + 1) * 8])
            if ck % 8 == 7:
                nc.scalar.activation(
                    st["wsb"][:, grp * 512:(grp + 1) * 512],
                    ps2[:], AF.Exp)

        TOTAL = NIT * NCK
        for gck in range(TOTAL + LAG):
            if gck < TOTAL:
                emit_l1(gck)
            if gck >= LAG:
                emit_l2(gck - LAG)
            if gck % NCK == 4 and gck // NCK >= 1 and gck < TOTAL:
                emit_av(gck // NCK - 1)
        emit_av(NIT - 1)

    nc.compile()
    return nc


_cache = {}


def kernel(**inputs):
    row_emb = np.asarray(inputs["row_emb"], dtype=np.float32)
    col_emb = np.asarray(inputs["col_emb"], dtype=np.float32)
    cost_mat = np.asarray(inputs["cost_mat"], dtype=np.float32)
    Wq = np.asarray(inputs["Wq"], dtype=np.float32)
    Wk = np.asarray(inputs["Wk"], dtype=np.float32)
    Wv = np.asarray(inputs["Wv"], dtype=np.float32)
    m1w = np.asarray(inputs["mix1_weight"], dtype=np.float32)
    m1b = np.asarray(inputs["mix1_bias"], dtype=np.float32)
    m2w = np.asarray(inputs["mix2_weight"], dtype=np.float32)

    a1 = m1w[:, 0, :]
    c1 = m1w[:, 1, :]
    w2 = m2w[:, :, 0]

    if "nc" not in _cache:
        _cache["nc"] = build_kernel()
    nc = _cache["nc"]

    wq_s = Wq * (1.0 / np.sqrt(D))

    w1l = np.zeros((17, 256), dtype=np.float32)
    w2l = np.zeros((128, 16), dtype=np.float32)
    bcol2 = np.zeros((128, 2), dtype=np.float32)
    for h in range(H):
        half, hl = h // 8, h % 8
        for m in range(MS):
            col = half * 128 + hl * 16 + m
            w1l[h, col] = a1[h, m]
            w1l[16, col] = c1[h, m]
            w2l[hl * 16 + m, half * 8 + hl] = w2[h, m]
            bcol2[hl * 16 + m, half] = m1b[h, m]

    in_maps = []
    for i in range(NCORES):
        sl = slice(i * BL, (i + 1) * BL)
        in_maps.append({
            "x_r": row_emb[sl].reshape(TOK, E),
            "x_c": col_emb[sl].reshape(TOK, E),
            "cost": cost_mat[sl],
            "Wq": wq_s, "Wk": Wk, "Wv": Wv,
            "W1L": w1l, "W2L": w2l, "bcol2": bcol2,
        })
    res = run_bass_kernel_spmd(nc, in_maps, list(range(NCORES)))
    out = np.concatenate([res.results[i]["out"] for i in range(NCORES)],
                         axis=0)
    return out.astype(np.float32)


# revision 34
# speedup vs baseline: 1.2124x; 1.0014x over previous
"""MixedScoreMultiHeadAttention Trainium2 kernel (v3: flat-pipelined evac).

Data-parallel over batch: 32 batches -> 8 cores x 4 batches.

Per core (4 batches):
  setup: batched input DMAs (x per-t so transposes start early), PE
         transposes, q/k projections, per-head dots -> x4all [r,(h,b,c)]
         fp16; the SBUF->SBUF gather DMAs for rhs[b] = [17, R*C] are
         interleaved with the dot evacs so main(b0) starts early.
  main: ONE flat stream over 128 global chunks (8 iterations of (b, half)
        x 16 chunks of 1024 pts):
    gck:      layer1 matmul pair -> ps_ev slot (3 PSUM bufs break the
              evac->L1 WAR chain), relu+bias evac alternating ACT/DVE
    gck-2:    layer2 matmuls of the chunk two back (possibly previous
              (b,half)) so the PE never head-blocks the next iteration
    grp ends: exp evac [128,512] ACT -> wsb
    gck%16==4: AV + reciprocal + broadcast-normalize of the PREVIOUS
              (b,half), placed where their deps are already satisfied.

The relu evacuation of H*MS*R*C*BL values (131072 partition-columns) through
the only two PSUM-capable engines (ACT ~0.99 col/ns, DVE ~0.86 col/ns in
1024-col chunks) is the fundamental floor (~71us); everything else is
arranged to keep those two engines saturated.

mix2 bias b2 is dropped (softmax-invariant); 1/sqrt(D) folded into Wq.
"""
import sys

sys.path.insert(0, "/opt/trn_rl_repo")

import numpy as np
from contextlib import ExitStack

import concourse.bass as bass
import concourse.mybir as mybir
import concourse.tile as tile
from concourse import bacc
from concourse.bass_utils import run_bass_kernel_spmd
from concourse.masks import make_identity

B, R, C, E, H, D, MS = 32, 128, 128, 256, 16, 16, 16
NCORES = 8
BL = B // NCORES  # batches per core: 4
TOK = BL * R      # 512 tokens per core per side
PTS = R * C       # 16384 score points per (b)

FP32 = mybir.dt.float32
FP16 = mybir.dt.float16
AF = mybir.ActivationFunctionType
ALU = mybir.AluOpType


def build_kernel():
    nc = bacc.Bacc("TRN2", target_bir_lowering=False, debug=False,
                   num_devices=NCORES)

    x_r = nc.dram_tensor("x_r", [TOK, E], FP32, kind="ExternalInput").ap()
    x_c = nc.dram_tensor("x_c", [TOK, E], FP32, kind="ExternalInput").ap()
    cost = nc.dram_tensor("cost", [BL, R, C], FP32, kind="ExternalInput").ap()
    wq_d = nc.dram_tensor("Wq", [E, E], FP32, kind="ExternalInput").ap()
    wk_d = nc.dram_tensor("Wk", [E, E], FP32, kind="ExternalInput").ap()
    wv_d = nc.dram_tensor("Wv", [E, E], FP32, kind="ExternalInput").ap()
    # layer1 stationary [17, 256]: col (half*128 + (h%8)*16 + m):
    #   row h' = a[h,m] iff h'==h; row 16 = c[h,m]
    w1_d = nc.dram_tensor("W1L", [17, 2 * 128], FP32,
                          kind="ExternalInput").ap()
    # layer2 moving [128, 16]: col (half*8 + j): row hm = w2[half*8+j, m]
    # iff hm == ((j)*16+m) else 0
    w2_d = nc.dram_tensor("W2L", [128, 16], FP32, kind="ExternalInput").ap()
    # relu bias per (h,m) row: bcol2[hm, half] = b1[half*8 + hm//16, hm%16]
    bc_d = nc.dram_tensor("bcol2", [128, 2], FP32, kind="ExternalInput").ap()
    out_d = nc.dram_tensor("out", [BL, R, H * D], FP32,
                           kind="ExternalOutput").ap()

    with tile.TileContext(nc) as tc, ExitStack() as ctx:
        const_p = ctx.enter_context(tc.tile_pool(name="const", bufs=1))
        stage_p = ctx.enter_context(tc.tile_pool(name="stage", bufs=1))
        xt_p = ctx.enter_context(tc.tile_pool(name="xt", bufs=1))
        w_p = ctx.enter_context(tc.tile_pool(name="wts", bufs=1))
        quad_p = ctx.enter_context(tc.tile_pool(name="quad", bufs=1))
        x4_p = ctx.enter_context(tc.tile_pool(name="x4", bufs=1))
        rhs_p = ctx.enter_context(tc.tile_pool(name="rhs", bufs=1))
        rr_p = ctx.enter_context(tc.tile_pool(name="rr", bufs=4))
        wsb_p = ctx.enter_context(tc.tile_pool(name="wsb", bufs=1))
        fout_p = ctx.enter_context(tc.tile_pool(name="fout", bufs=1))
        small_p = ctx.enter_context(tc.tile_pool(name="small", bufs=2))
        # PSUM (8 banks): ev 3x[128,1024] = 6 banks, l2 1x[128,512] = 1,
        # av 1x[128,144] = 1
        ps_ev = ctx.enter_context(
            tc.tile_pool(name="psev", bufs=3, space="PSUM"))
        ps_l2 = ctx.enter_context(
            tc.tile_pool(name="psl2", bufs=1, space="PSUM"))
        ps_av = ctx.enter_context(
            tc.tile_pool(name="psav", bufs=1, space="PSUM"))

        # round-robin assignment of setup evac work to the two PSUM engines
        eng_i = [0]

        def evac_engine():
            eng_i[0] ^= 1
            return nc.scalar if eng_i[0] else nc.vector

        def evac_copy(dst, src):
            ev = evac_engine()
            (ev.copy if ev is nc.scalar else ev.tensor_copy)(dst, src)

        # ---- input DMAs (SP queue) ----
        w1f = stage_p.tile([17, 256], FP32, tag="w1f", name="w1f")
        nc.sync.dma_start(w1f[:], w1_d[:])
        w2f = stage_p.tile([128, 16], FP32, tag="w2f", name="w2f")
        nc.sync.dma_start(w2f[:], w2_d[:])
        bcol2 = const_p.tile([128, 2], FP32)
        nc.sync.dma_start(bcol2[:], bc_d[:])

        wdram = {"q": wq_d, "k": wk_d, "v": wv_d}

        def load_w32(name):
            w = stage_p.tile([128, 2, E], FP32, tag="w32", bufs=2,
                             name=f"w32{name}")
            nc.sync.dma_start(
                w[:], wdram[name][:].rearrange("(eh p) e -> p eh e", p=128))
            return w

        # x first on the SP queue (the transpose chain gates setup); one
        # batched DMA per side -- HWDGE pays 625ns fixed per DMA
        x32 = {}
        for name, dram in (("r", x_r), ("c", x_c)):
            x32[name] = stage_p.tile([128, BL, E], FP32, tag=f"x32{name}",
                                     name=f"x32{name}")
            nc.sync.dma_start(
                x32[name][:], dram[:].rearrange("(t p) e -> p t e", p=128))
        xsl = {name: [x32[name][:, t, :] for t in range(BL)]
               for name in ("r", "c")}

        cost32 = stage_p.tile([128, BL, C], FP32, tag="cost32", name="cost32")
        nc.sync.dma_start(cost32[:], cost[:].rearrange("b r c -> r b c"))

        # ---- const/weight prep ----
        ident = const_p.tile([128, 128], FP32)
        make_identity(nc, ident[:])

        w1l = const_p.tile([17, 256], FP16)
        nc.gpsimd.tensor_copy(w1l[:], w1f[:])
        w2l = const_p.tile([128, 16], FP16)
        nc.gpsimd.tensor_copy(w2l[:], w2f[:])

        # q/k weights padded on-chip: head h -> 32-col slot; v unpadded
        wt16 = {}
        for name in ("q", "k"):
            w32 = load_w32(name)
            wt = w_p.tile([128, 2, 2 * E], FP16, tag=f"wt{name}",
                          name=f"wt{name}")
            nc.gpsimd.memset(wt[:], 0.0)
            wt4 = wt[:].rearrange("p eh (h x) -> p eh h x", h=H)
            w4 = w32[:].rearrange("p eh (h d) -> p eh h d", h=H)
            nc.gpsimd.tensor_copy(wt4[:, :, :, 0:D], w4[:])
            wt16[name] = wt
        w32v = load_w32("v")
        wtv = w_p.tile([128, 2, E], FP16, tag="wtv", name="wtv")
        nc.gpsimd.tensor_copy(wtv[:], w32v[:])

        y16all = const_p.tile([128, BL * C], FP16, name="y16all")
        nc.gpsimd.tensor_copy(y16all[:], cost32[:].rearrange("p b c -> p (b c)"))

        # ---- PE transposes of the x slices ----
        xT = {}
        for name in ("r", "c"):
            xT[name] = [xt_p.tile([128, TOK], FP16, tag=f"xT{name}{eh}",
                                  name=f"xT{name}{eh}") for eh in range(2)]
            for eh in range(2):
                ps = ps_ev.tile([128, 1024], FP32, tag="ev", name="psev")
                for t in range(BL):
                    nc.tensor.transpose(
                        ps[:, t * 128:(t + 1) * 128],
                        xsl[name][t][:, eh * 128:(eh + 1) * 128], ident[:])
                evac_copy(xT[name][eh][:], ps[:, 0:512])

        # ---- projections + dots + gather, interleaved per quad-pair ----
        # (PE matmul operands must sit at base partition 0/32/64; 96 is
        #  invalid, so the 4th head of each quad lives in a base-0 tile)
        x4all = x4_p.tile([128, H * BL * C], FP16, name="x4all")
        x4v = x4all[:].rearrange("p (h b c) -> p h b c", h=H, b=BL)
        # single gathered rhs, b-major pts: [17, (b, r, c)] fp16.
        # One DMA per (head, b) keeps each DMA cheap in the tile scheduler's
        # internal (byte-cost-dominated) DMA model so later batches'
        # gathers aren't chained too deep behind main-loop progress.
        rhs_all = rhs_p.tile([17, BL * PTS], FP16, tag="rhs", name="rhs")

        def gather_h(h, b):
            nc.sync.dma_start(
                rhs_all[h:h + 1, b * PTS:(b + 1) * PTS], x4v[:, h, b, :])

        # head-GROUPS of 3 (heads at psum rows 0/32/64 -- all valid matmul
        # base partitions, unlike 96): 6 groups of (3,3,3,3,3,1) heads
        GSZ = [3, 3, 3, 3, 3, 1]
        GOF = [0, 3, 6, 9, 12, 15]
        qAll = {"q": quad_p.tile([96, 6 * TOK], FP16, tag="qAll",
                                 name="qAll"),
                "k": quad_p.tile([96, 6 * TOK], FP16, tag="kAll",
                                 name="kAll")}

        def proj_groups(proj, gp, dst):
            # two 3-head groups share one [128,1024] psum slot
            ps = ps_ev.tile([128, 2 * TOK], FP32, tag="ev", name="psev")
            for i in range(2):
                g = gp * 2 + i
                for eh in range(2):
                    nc.tensor.matmul(
                        ps[0:32 * GSZ[g], i * TOK:(i + 1) * TOK],
                        wt16[proj][:, eh, GOF[g] * 32:
                                   (GOF[g] + GSZ[g]) * 32],
                        xT["r" if proj == "q" else "c"][eh][:],
                        start=(eh == 0), stop=(eh == 1))
            for i in range(2):
                g = gp * 2 + i
                evac_copy(qAll[proj][0:32 * GSZ[g],
                                     g * TOK:(g + 1) * TOK],
                          ps[0:32 * GSZ[g], i * TOK:(i + 1) * TOK])

        def head_slice(proj, h):
            g, loc = h // 3, h % 3
            return qAll[proj][loc * 32:loc * 32 + 16,
                              g * TOK:(g + 1) * TOK]

        qT = [head_slice("q", h) for h in range(H)]
        kT = [head_slice("k", h) for h in range(H)]
        for gp in range(3):
            proj_groups("q", gp, None)
            proj_groups("k", gp, None)
            for hp in range(3 * gp, min(3 * gp + 3, 8)):  # head pairs
                ps = ps_ev.tile([128, 1024], FP32, tag="ev", name="psev")
                for i in range(2):
                    h = hp * 2 + i
                    for b in range(BL):
                        nc.tensor.matmul(
                            ps[:, i * 512 + b * 128:i * 512 + (b + 1) * 128],
                            qT[h][:, b * 128:(b + 1) * 128],
                            kT[h][:, b * 128:(b + 1) * 128])
                evac_copy(x4all[:, hp * 1024:(hp + 1) * 1024], ps[:])
                gather_h(hp * 2, 0)
                gather_h(hp * 2 + 1, 0)
        # cost row b0, then remaining batches (overlap the main loop)
        nc.sync.dma_start(rhs_all[16:17, 0:PTS], y16all[:, 0:128])
        for b in range(1, BL):
            for h in range(H):
                gather_h(h, b)
            nc.sync.dma_start(
                rhs_all[16:17, b * PTS:(b + 1) * PTS],
                y16all[:, b * 128:(b + 1) * 128])

        # ---- v projections (first needed at AV(b0), well into main) ----
        vhat = []
        for b in range(BL):
            vh = quad_p.tile([128, 17 * H], FP32, tag=f"vhat{b}",
                             name=f"vhat{b}")
            vh3 = vh[:].rearrange("p (h x) -> p h x", h=H)
            nc.gpsimd.memset(vh3[:, :, 16:17], 1.0)
            ps = ps_ev.tile([128, 1024], FP32, tag="ev", name="psev")
            for eh in range(2):
                nc.tensor.matmul(
                    ps[:, 0:E], xT["c"][eh][:, b * 128:(b + 1) * 128],
                    wtv[:, eh, :], start=(eh == 0), stop=(eh == 1))
            evac_copy(vh3[:, :, 0:16],
                      ps[:, 0:E].rearrange("p (h d) -> p h d", h=H))
            vhat.append(vh)

        # ---- main loop: flat stream over 128 global chunks ----
        fouts = [fout_p.tile([128, H * D], FP32, tag=f"fo{b}", name=f"fo{b}")
                 for b in range(BL)]

        NIT = 2 * BL          # 8 (b, half) iterations
        NCK = 16              # chunks per iteration
        LAG = 2               # layer2 lag in chunks
        state = {}            # per-iteration: ps2, wsb, rr chunks

        def emit_av(it):
            b, half = it // 2, it % 2
            wsb = state[it]["wsb"]
            psa_t = ps_av.tile([128, 144], FP32, tag="av", name="psa_t")
            psa = psa_t[:, 0:17 * 8]
            wsb3 = wsb[:].rearrange("p (s h) -> p s h", h=8)
            for hl in range(8):
                h = half * 8 + hl
                nc.tensor.matmul(
                    psa[:, hl * 17:(hl + 1) * 17],
                    wsb3[:, :, hl],
                    vhat[b][:, h * 17:(h + 1) * 17])
            psa3 = psa.rearrange("p (x y) -> p x y", x=8)
            rec = small_p.tile([128, 8], FP32, tag="rec", name="rec")
            nc.vector.reciprocal(rec[:], psa3[:, :, 16])
            fo3 = fouts[b][:, half * 128:(half + 1) * 128].rearrange(
                "p (x y) -> p x y", x=8)
            nc.vector.tensor_tensor(
                fo3[:], psa3[:, :, 0:16],
                rec[:].broadcast_to((128, 8, 16)), ALU.mult)
            # ship each half as soon as it is normalized: the final
            # iteration's out-DMA then moves only 128 cols off the tail
            nc.sync.dma_start(
                out_d[b][:, half * 128:(half + 1) * 128],
                fouts[b][:, half * 128:(half + 1) * 128])

        def emit_l1(gck):
            it, ck = gck // NCK, gck % NCK
            b, half = it // 2, it % 2
            if ck == 0:
                state[it] = {
                    "wsb": wsb_p.tile([128, 1024], FP32, tag="wsb",
                                      name="wsb"),
                    "ps2": [None, None],
                    "rr": {},
                }
            ps = ps_ev.tile([128, 1024], FP32, tag="ev", name="psev")
            base = b * PTS + ck * 1024
            for j in range(2):
                nc.tensor.matmul(
                    ps[:, j * 512:(j + 1) * 512],
                    w1l[:, half * 128:(half + 1) * 128],
                    rhs_all[:, base + j * 512:base + (j + 1) * 512])
            rr = rr_p.tile([128, 1024], FP16, tag="rr", name="rr")
            to_act = ck % 2 == 0
            if to_act:
                nc.scalar.activation(rr[:], ps[:], AF.Relu,
                                     bias=bcol2[:, half:half + 1])
            else:
                nc.vector.tensor_scalar(rr[:], ps[:],
                                        bcol2[:, half:half + 1],
                                        0.0, ALU.add, ALU.max)
            state[it]["rr"][ck] = rr

        def emit_l2(gck):
            it, ck = gck // NCK, gck % NCK
            half = it % 2
            st = state[it]
            grp = ck // 8
            if ck % 8 == 0:
                st["ps2"][grp] = ps_l2.tile([128, 512], FP32, tag="l2",
                                           name="ps2")
            ps2 = st["ps2"][grp]
            rr = st["rr"].pop(ck)
            for s in range(8):
                rloc = (ck % 8) * 8 + s
                nc.tensor.matmul(
                    ps2[:, rloc * 8:rloc * 8 + 8],
                    rr[:, s * 128:(s + 1) * 128],
                    w2l[:, half * 8:(half + 1) * 8])
            if ck % 8 == 7:
                nc.scalar.activation(
                    st["wsb"][:, grp * 512:(grp + 1) * 512],
                    ps2[:], AF.Exp)

        TOTAL = NIT * NCK
        for gck in range(TOTAL + LAG):
            if gck < TOTAL:
                emit_l1(gck)
            if gck >= LAG:
                emit_l2(gck - LAG)
            if gck % NCK == 4 and gck // NCK >= 1 and gck < TOTAL:
                emit_av(gck // NCK - 1)
        emit_av(NIT - 1)

    nc.compile()
    return nc


_cache = {}


def kernel(**inputs):
    row_emb = np.asarray(inputs["row_emb"], dtype=np.float32)
    col_emb = np.asarray(inputs["col_emb"], dtype=np.float32)
    cost_mat = np.asarray(inputs["cost_mat"], dtype=np.float32)
    Wq = np.asarray(inputs["Wq"], dtype=np.float32)
    Wk = np.asarray(inputs["Wk"], dtype=np.float32)
    Wv = np.asarray(inputs["Wv"], dtype=np.float32)
    m1w = np.asarray(inputs["mix1_weight"], dtype=np.float32)
    m1b = np.asarray(inputs["mix1_bias"], dtype=np.float32)
    m2w = np.asarray(inputs["mix2_weight"], dtype=np.float32)

    a1 = m1w[:, 0, :]
    c1 = m1w[:, 1, :]
    w2 = m2w[:, :, 0]

    if "nc" not in _cache:
        _cache["nc"] = build_kernel()
    nc = _cache["nc"]

    wq_s = Wq * (1.0 / np.sqrt(D))

    w1l = np.zeros((17, 256), dtype=np.float32)
    w2l = np.zeros((128, 16), dtype=np.float32)
    bcol2 = np.zeros((128, 2), dtype=np.float32)
    for h in range(H):
        half, hl = h // 8, h % 8
        for m in range(MS):
            col = half * 128 + hl * 16 + m
            w1l[h, col] = a1[h, m]
            w1l[16, col] = c1[h, m]
            w2l[hl * 16 + m, half * 8 + hl] = w2[h, m]
            bcol2[hl * 16 + m, half] = m1b[h, m]

    in_maps = []
    for i in range(NCORES):
        sl = slice(i * BL, (i + 1) * BL)
        in_maps.append({
            "x_r": row_emb[sl].reshape(TOK, E),
            "x_c": col_emb[sl].reshape(TOK, E),
            "cost": cost_mat[sl],
            "Wq": wq_s, "Wk": Wk, "Wv": Wv,
            "W1L": w1l, "W2L": w2l, "bcol2": bcol2,
        })
    res = run_bass_kernel_spmd(nc, in_maps, list(range(NCORES)))
    out = np.concatenate([res.results[i]["out"] for i in range(NCORES)],
                         axis=0)
    return out.astype(np.float32)


# revision 35
# speedup vs baseline: 1.2154x; 1.0024x over previous
"""MixedScoreMultiHeadAttention Trainium2 kernel (v3: flat-pipelined evac).

Data-parallel over batch: 32 batches -> 8 cores x 4 batches.

Per core (4 batches):
  setup: batched input DMAs (x per-t so transposes start early), PE
         transposes, q/k projections, per-head dots -> x4all [r,(h,b,c)]
         fp16; the SBUF->SBUF gather DMAs for rhs[b] = [17, R*C] are
         interleaved with the dot evacs so main(b0) starts early.
  main: ONE flat stream over 128 global chunks (8 iterations of (b, half)
        x 16 chunks of 1024 pts):
    gck:      layer1 matmul pair -> ps_ev slot (3 PSUM bufs break the
              evac->L1 WAR chain), relu+bias evac alternating ACT/DVE
    gck-2:    layer2 matmuls of the chunk two back (possibly previous
              (b,half)) so the PE never head-blocks the next iteration
    grp ends: exp evac [128,512] ACT -> wsb
    gck%16==4: AV + reciprocal + broadcast-normalize of the PREVIOUS
              (b,half), placed where their deps are already satisfied.

The relu evacuation of H*MS*R*C*BL values (131072 partition-columns) through
the only two PSUM-capable engines (ACT ~0.99 col/ns, DVE ~0.86 col/ns in
1024-col chunks) is the fundamental floor (~71us); everything else is
arranged to keep those two engines saturated.

mix2 bias b2 is dropped (softmax-invariant); 1/sqrt(D) folded into Wq.
"""
import sys

sys.path.insert(0, "/opt/trn_rl_repo")

import numpy as np
from contextlib import ExitStack

import concourse.bass as bass
import concourse.mybir as mybir
import concourse.tile as tile
from concourse import bacc
from concourse.bass_utils import run_bass_kernel_spmd
from concourse.masks import make_identity

B, R, C, E, H, D, MS = 32, 128, 128, 256, 16, 16, 16
NCORES = 8
BL = B // NCORES  # batches per core: 4
TOK = BL * R      # 512 tokens per core per side
PTS = R * C       # 16384 score points per (b)

FP32 = mybir.dt.float32
FP16 = mybir.dt.float16
AF = mybir.ActivationFunctionType
ALU = mybir.AluOpType


def build_kernel():
    nc = bacc.Bacc("TRN2", target_bir_lowering=False, debug=False,
                   num_devices=NCORES)

    x_r = nc.dram_tensor("x_r", [TOK, E], FP32, kind="ExternalInput").ap()
    x_c = nc.dram_tensor("x_c", [TOK, E], FP32, kind="ExternalInput").ap()
    cost = nc.dram_tensor("cost", [BL, R, C], FP32, kind="ExternalInput").ap()
    wq_d = nc.dram_tensor("Wq", [E, E], FP32, kind="ExternalInput").ap()
    wk_d = nc.dram_tensor("Wk", [E, E], FP32, kind="ExternalInput").ap()
    wv_d = nc.dram_tensor("Wv", [E, E], FP32, kind="ExternalInput").ap()
    # layer1 stationary [17, 256]: col (half*128 + (h%8)*16 + m):
    #   row h' = a[h,m] iff h'==h; row 16 = c[h,m]
    w1_d = nc.dram_tensor("W1L", [17, 2 * 128], FP32,
                          kind="ExternalInput").ap()
    # layer2 moving [128, 16]: col (half*8 + j): row hm = w2[half*8+j, m]
    # iff hm == ((j)*16+m) else 0
    w2_d = nc.dram_tensor("W2L", [128, 16], FP32, kind="ExternalInput").ap()
    # relu bias per (h,m) row: bcol2[hm, half] = b1[half*8 + hm//16, hm%16]
    bc_d = nc.dram_tensor("bcol2", [128, 2], FP32, kind="ExternalInput").ap()
    out_d = nc.dram_tensor("out", [BL, R, H * D], FP32,
                           kind="ExternalOutput").ap()

    with tile.TileContext(nc) as tc, ExitStack() as ctx:
        const_p = ctx.enter_context(tc.tile_pool(name="const", bufs=1))
        stage_p = ctx.enter_context(tc.tile_pool(name="stage", bufs=1))
        xt_p = ctx.enter_context(tc.tile_pool(name="xt", bufs=1))
        w_p = ctx.enter_context(tc.tile_pool(name="wts", bufs=1))
        quad_p = ctx.enter_context(tc.tile_pool(name="quad", bufs=1))
        x4_p = ctx.enter_context(tc.tile_pool(name="x4", bufs=1))
        rhs_p = ctx.enter_context(tc.tile_pool(name="rhs", bufs=1))
        rr_p = ctx.enter_context(tc.tile_pool(name="rr", bufs=4))
        wsb_p = ctx.enter_context(tc.tile_pool(name="wsb", bufs=1))
        fout_p = ctx.enter_context(tc.tile_pool(name="fout", bufs=1))
        small_p = ctx.enter_context(tc.tile_pool(name="small", bufs=2))
        # PSUM (8 banks): ev 3x[128,1024] = 6 banks, l2 1x[128,512] = 1,
        # av 1x[128,144] = 1
        ps_ev = ctx.enter_context(
            tc.tile_pool(name="psev", bufs=3, space="PSUM"))
        ps_l2 = ctx.enter_context(
            tc.tile_pool(name="psl2", bufs=1, space="PSUM"))
        ps_av = ctx.enter_context(
            tc.tile_pool(name="psav", bufs=1, space="PSUM"))

        # round-robin assignment of setup evac work to the two PSUM engines
        eng_i = [0]

        def evac_engine():
            eng_i[0] ^= 1
            return nc.scalar if eng_i[0] else nc.vector

        def evac_copy(dst, src):
            ev = evac_engine()
            (ev.copy if ev is nc.scalar else ev.tensor_copy)(dst, src)

        # ---- input DMAs (SP queue): x absolutely first (the transpose
        # chain gates all of setup; HWDGE serializes at 625ns per DMA),
        # then q/k weights (needed at proj), everything else after ----
        x32 = {}
        for name, dram in (("r", x_r), ("c", x_c)):
            x32[name] = stage_p.tile([128, BL, E], FP32, tag=f"x32{name}",
                                     name=f"x32{name}")
            nc.sync.dma_start(
                x32[name][:], dram[:].rearrange("(t p) e -> p t e", p=128))
        xsl = {name: [x32[name][:, t, :] for t in range(BL)]
               for name in ("r", "c")}

        wdram = {"q": wq_d, "k": wk_d, "v": wv_d}

        def load_w32(name):
            w = stage_p.tile([128, 2, E], FP32, tag="w32", bufs=2,
                             name=f"w32{name}")
            nc.sync.dma_start(
                w[:], wdram[name][:].rearrange("(eh p) e -> p eh e", p=128))
            return w

        w32q = load_w32("q")
        w32k = load_w32("k")

        cost32 = stage_p.tile([128, BL, C], FP32, tag="cost32", name="cost32")
        nc.sync.dma_start(cost32[:], cost[:].rearrange("b r c -> r b c"))
        w1f = stage_p.tile([17, 256], FP32, tag="w1f", name="w1f")
        nc.sync.dma_start(w1f[:], w1_d[:])
        w2f = stage_p.tile([128, 16], FP32, tag="w2f", name="w2f")
        nc.sync.dma_start(w2f[:], w2_d[:])
        bcol2 = const_p.tile([128, 2], FP32)
        nc.sync.dma_start(bcol2[:], bc_d[:])

        # ---- const/weight prep ----
        ident = const_p.tile([128, 128], FP32)
        make_identity(nc, ident[:])

        w1l = const_p.tile([17, 256], FP16)
        nc.gpsimd.tensor_copy(w1l[:], w1f[:])
        w2l = const_p.tile([128, 16], FP16)
        nc.gpsimd.tensor_copy(w2l[:], w2f[:])

        # q/k weights padded on-chip: head h -> 32-col slot; v unpadded
        wt16 = {}
        for name, w32 in (("q", w32q), ("k", w32k)):
            wt = w_p.tile([128, 2, 2 * E], FP16, tag=f"wt{name}",
                          name=f"wt{name}")
            nc.gpsimd.memset(wt[:], 0.0)
            wt4 = wt[:].rearrange("p eh (h x) -> p eh h x", h=H)
            w4 = w32[:].rearrange("p eh (h d) -> p eh h d", h=H)
            nc.gpsimd.tensor_copy(wt4[:, :, :, 0:D], w4[:])
            wt16[name] = wt
        w32v = load_w32("v")
        wtv = w_p.tile([128, 2, E], FP16, tag="wtv", name="wtv")
        nc.gpsimd.tensor_copy(wtv[:], w32v[:])

        y16all = const_p.tile([128, BL * C], FP16, name="y16all")
        nc.gpsimd.tensor_copy(y16all[:], cost32[:].rearrange("p b c -> p (b c)"))

        # ---- PE transposes of the x slices ----
        xT = {}
        for name in ("r", "c"):
            xT[name] = [xt_p.tile([128, TOK], FP16, tag=f"xT{name}{eh}",
                                  name=f"xT{name}{eh}") for eh in range(2)]
            for eh in range(2):
                ps = ps_ev.tile([128, 1024], FP32, tag="ev", name="psev")
                for t in range(BL):
                    nc.tensor.transpose(
                        ps[:, t * 128:(t + 1) * 128],
                        xsl[name][t][:, eh * 128:(eh + 1) * 128], ident[:])
                evac_copy(xT[name][eh][:], ps[:, 0:512])

        # ---- projections + dots + gather, interleaved per quad-pair ----
        # (PE matmul operands must sit at base partition 0/32/64; 96 is
        #  invalid, so the 4th head of each quad lives in a base-0 tile)
        x4all = x4_p.tile([128, H * BL * C], FP16, name="x4all")
        x4v = x4all[:].rearrange("p (h b c) -> p h b c", h=H, b=BL)
        # single gathered rhs, b-major pts: [17, (b, r, c)] fp16.
        # One DMA per (head, b) keeps each DMA cheap in the tile scheduler's
        # internal (byte-cost-dominated) DMA model so later batches'
        # gathers aren't chained too deep behind main-loop progress.
        rhs_all = rhs_p.tile([17, BL * PTS], FP16, tag="rhs", name="rhs")

        def gather_h(h, b):
            nc.sync.dma_start(
                rhs_all[h:h + 1, b * PTS:(b + 1) * PTS], x4v[:, h, b, :])

        # head-GROUPS of 3 (heads at psum rows 0/32/64 -- all valid matmul
        # base partitions, unlike 96): 6 groups of (3,3,3,3,3,1) heads
        GSZ = [3, 3, 3, 3, 3, 1]
        GOF = [0, 3, 6, 9, 12, 15]
        qAll = {"q": quad_p.tile([96, 6 * TOK], FP16, tag="qAll",
                                 name="qAll"),
                "k": quad_p.tile([96, 6 * TOK], FP16, tag="kAll",
                                 name="kAll")}

        def proj_groups(proj, gp, dst):
            # two 3-head groups share one [128,1024] psum slot
            ps = ps_ev.tile([128, 2 * TOK], FP32, tag="ev", name="psev")
            for i in range(2):
                g = gp * 2 + i
                for eh in range(2):
                    nc.tensor.matmul(
                        ps[0:32 * GSZ[g], i * TOK:(i + 1) * TOK],
                        wt16[proj][:, eh, GOF[g] * 32:
                                   (GOF[g] + GSZ[g]) * 32],
                        xT["r" if proj == "q" else "c"][eh][:],
                        start=(eh == 0), stop=(eh == 1))
            for i in range(2):
                g = gp * 2 + i
                evac_copy(qAll[proj][0:32 * GSZ[g],
                                     g * TOK:(g + 1) * TOK],
                          ps[0:32 * GSZ[g], i * TOK:(i + 1) * TOK])

        def head_slice(proj, h):
            g, loc = h // 3, h % 3
            return qAll[proj][loc * 32:loc * 32 + 16,
                              g * TOK:(g + 1) * TOK]

        qT = [head_slice("q", h) for h in range(H)]
        kT = [head_slice("k", h) for h in range(H)]
        for gp in range(3):
            proj_groups("q", gp, None)
            proj_groups("k", gp, None)
            for hp in range(3 * gp, min(3 * gp + 3, 8)):  # head pairs
                ps = ps_ev.tile([128, 1024], FP32, tag="ev", name="psev")
                for i in range(2):
                    h = hp * 2 + i
                    for b in range(BL):
                        nc.tensor.matmul(
                            ps[:, i * 512 + b * 128:i * 512 + (b + 1) * 128],
                            qT[h][:, b * 128:(b + 1) * 128],
                            kT[h][:, b * 128:(b + 1) * 128])
                evac_copy(x4all[:, hp * 1024:(hp + 1) * 1024], ps[:])
                gather_h(hp * 2, 0)
                gather_h(hp * 2 + 1, 0)
        # cost row b0, then remaining batches (overlap the main loop)
        nc.sync.dma_start(rhs_all[16:17, 0:PTS], y16all[:, 0:128])
        for b in range(1, BL):
            for h in range(H):
                gather_h(h, b)
            nc.sync.dma_start(
                rhs_all[16:17, b * PTS:(b + 1) * PTS],
                y16all[:, b * 128:(b + 1) * 128])

        # ---- v projections (first needed at AV(b0), well into main) ----
        vhat = []
        for b in range(BL):
            vh = quad_p.tile([128, 17 * H], FP32, tag=f"vhat{b}",
                             name=f"vhat{b}")
            vh3 = vh[:].rearrange("p (h x) -> p h x", h=H)
            nc.gpsimd.memset(vh3[:, :, 16:17], 1.0)
            ps = ps_ev.tile([128, 1024], FP32, tag="ev", name="psev")
            for eh in range(2):
                nc.tensor.matmul(
                    ps[:, 0:E], xT["c"][eh][:, b * 128:(b + 1) * 128],
                    wtv[:, eh, :], start=(eh == 0), stop=(eh == 1))
            evac_copy(vh3[:, :, 0:16],
                      ps[:, 0:E].rearrange("p (h d) -> p h d", h=H))
            vhat.append(vh)

        # ---- main loop: flat stream over 128 global chunks ----
        fouts = [fout_p.tile([128, H * D], FP32, tag=f"fo{b}", name=f"fo{b}")
                 for b in range(BL)]

        NIT = 2 * BL          # 8 (b, half) iterations
        NCK = 16              # chunks per iteration
        LAG = 2               # layer2 lag in chunks
        state = {}            # per-iteration: ps2, wsb, rr chunks

        def emit_av(it):
            b, half = it // 2, it % 2
            wsb = state[it]["wsb"]
            psa_t = ps_av.tile([128, 144], FP32, tag="av", name="psa_t")
            psa = psa_t[:, 0:17 * 8]
            wsb3 = wsb[:].rearrange("p (s h) -> p s h", h=8)
            for hl in range(8):
                h = half * 8 + hl
                nc.tensor.matmul(
                    psa[:, hl * 17:(hl + 1) * 17],
                    wsb3[:, :, hl],
                    vhat[b][:, h * 17:(h + 1) * 17])
            psa3 = psa.rearrange("p (x y) -> p x y", x=8)
            rec = small_p.tile([128, 8], FP32, tag="rec", name="rec")
            nc.vector.reciprocal(rec[:], psa3[:, :, 16])
            fo3 = fouts[b][:, half * 128:(half + 1) * 128].rearrange(
                "p (x y) -> p x y", x=8)
            nc.vector.tensor_tensor(
                fo3[:], psa3[:, :, 0:16],
                rec[:].broadcast_to((128, 8, 16)), ALU.mult)
            # ship each half as soon as it is normalized: the final
            # iteration's out-DMA then moves only 128 cols off the tail
            nc.sync.dma_start(
                out_d[b][:, half * 128:(half + 1) * 128],
                fouts[b][:, half * 128:(half + 1) * 128])

        def emit_l1(gck):
            it, ck = gck // NCK, gck % NCK
            b, half = it // 2, it % 2
            if ck == 0:
                state[it] = {
                    "wsb": wsb_p.tile([128, 1024], FP32, tag="wsb",
                                      name="wsb"),
                    "ps2": [None, None],
                    "rr": {},
                }
            ps = ps_ev.tile([128, 1024], FP32, tag="ev", name="psev")
            base = b * PTS + ck * 1024
            for j in range(2):
                nc.tensor.matmul(
                    ps[:, j * 512:(j + 1) * 512],
                    w1l[:, half * 128:(half + 1) * 128],
                    rhs_all[:, base + j * 512:base + (j + 1) * 512])
            rr = rr_p.tile([128, 1024], FP16, tag="rr", name="rr")
            to_act = ck % 2 == 0
            if to_act:
                nc.scalar.activation(rr[:], ps[:], AF.Relu,
                                     bias=bcol2[:, half:half + 1])
            else:
                nc.vector.tensor_scalar(rr[:], ps[:],
                                        bcol2[:, half:half + 1],
                                        0.0, ALU.add, ALU.max)
            state[it]["rr"][ck] = rr

        def emit_l2(gck):
            it, ck = gck // NCK, gck % NCK
            half = it % 2
            st = state[it]
            grp = ck // 8
            if ck % 8 == 0:
                st["ps2"][grp] = ps_l2.tile([128, 512], FP32, tag="l2",
                                           name="ps2")
            ps2 = st["ps2"][grp]
            rr = st["rr"].pop(ck)
            for s in range(8):
                rloc = (ck % 8) * 8 + s
                nc.tensor.matmul(
                    ps2[:, rloc * 8:rloc * 8 + 8],
                    rr[:, s * 128:(s + 1) * 128],
                    w2l[:, half * 8:(half + 1) * 8])
            if ck % 8 == 7:
                nc.scalar.activation(
                    st["wsb"][:, grp * 512:(grp + 1) * 512],
                    ps2[:], AF.Exp)

        TOTAL = NIT * NCK
        for gck in range(TOTAL + LAG):
            if gck < TOTAL:
                emit_l1(gck)
            if gck >= LAG:
                emit_l2(gck - LAG)
            if gck % NCK == 4 and gck // NCK >= 1 and gck < TOTAL:
                emit_av(gck // NCK - 1)
        emit_av(NIT - 1)

    nc.compile()
    return nc


_cache = {}


def kernel(**inputs):
    row_emb = np.asarray(inputs["row_emb"], dtype=np.float32)
    col_emb = np.asarray(inputs["col_emb"], dtype=np.float32)
    cost_mat = np.asarray(inputs["cost_mat"], dtype=np.float32)
    Wq = np.asarray(inputs["Wq"], dtype=np.float32)
    Wk = np.asarray(inputs["Wk"], dtype=np.float32)
    Wv = np.asarray(inputs["Wv"], dtype=np.float32)
    m1w = np.asarray(inputs["mix1_weight"], dtype=np.float32)
    m1b = np.asarray(inputs["mix1_bias"], dtype=np.float32)
    m2w = np.asarray(inputs["mix2_weight"], dtype=np.float32)

    a1 = m1w[:, 0, :]
    c1 = m1w[:, 1, :]
    w2 = m2w[:, :, 0]

    if "nc" not in _cache:
        _cache["nc"] = build_kernel()
    nc = _cache["nc"]

    wq_s = Wq * (1.0 / np.sqrt(D))

    w1l = np.zeros((17, 256), dtype=np.float32)
    w2l = np.zeros((128, 16), dtype=np.float32)
    bcol2 = np.zeros((128, 2), dtype=np.float32)
    for h in range(H):
        half, hl = h // 8, h % 8
        for m in range(MS):
            col = half * 128 + hl * 16 + m
            w1l[h, col] = a1[h, m]
            w1l[16, col] = c1[h, m]
            w2l[hl * 16 + m, half * 8 + hl] = w2[h, m]
            bcol2[hl * 16 + m, half] = m1b[h, m]

    in_maps = []
    for i in range(NCORES):
        sl = slice(i * BL, (i + 1) * BL)
        in_maps.append({
            "x_r": row_emb[sl].reshape(TOK, E),
            "x_c": col_emb[sl].reshape(TOK, E),
            "cost": cost_mat[sl],
            "Wq": wq_s, "Wk": Wk, "Wv": Wv,
            "W1L": w1l, "W2L": w2l, "bcol2": bcol2,
        })
    res = run_bass_kernel_spmd(nc, in_maps, list(range(NCORES)))
    out = np.concatenate([res.results[i]["out"] for i in range(NCORES)],
                         axis=0)
    return out.astype(np.float32)


# revision 36
# speedup vs baseline: 1.2333x; 1.0147x over previous
"""MixedScoreMultiHeadAttention Trainium2 kernel (v3: flat-pipelined evac).

Data-parallel over batch: 32 batches -> 8 cores x 4 batches.

Per core (4 batches):
  setup: batched input DMAs (x per-t so transposes start early), PE
         transposes, q/k projections, per-head dots -> x4all [r,(h,b,c)]
         fp16; the SBUF->SBUF gather DMAs for rhs[b] = [17, R*C] are
         interleaved with the dot evacs so main(b0) starts early.
  main: ONE flat stream over 128 global chunks (8 iterations of (b, half)
        x 16 chunks of 1024 pts):
    gck:      layer1 matmul pair -> ps_ev slot (3 PSUM bufs break the
              evac->L1 WAR chain), relu+bias evac alternating ACT/DVE
    gck-2:    layer2 matmuls of the chunk two back (possibly previous
              (b,half)) so the PE never head-blocks the next iteration
    grp ends: exp evac [128,512] ACT -> wsb
    gck%16==4: AV + reciprocal + broadcast-normalize of the PREVIOUS
              (b,half), placed where their deps are already satisfied.

The relu evacuation of H*MS*R*C*BL values (131072 partition-columns) through
the only two PSUM-capable engines (ACT ~0.99 col/ns, DVE ~0.86 col/ns in
1024-col chunks) is the fundamental floor (~71us); everything else is
arranged to keep those two engines saturated.

mix2 bias b2 is dropped (softmax-invariant); 1/sqrt(D) folded into Wq.
"""
import sys

sys.path.insert(0, "/opt/trn_rl_repo")

import numpy as np
from contextlib import ExitStack

import concourse.bass as bass
import concourse.mybir as mybir
import concourse.tile as tile
from concourse import bacc
from concourse.bass_utils import run_bass_kernel_spmd
from concourse.masks import make_identity

B, R, C, E, H, D, MS = 32, 128, 128, 256, 16, 16, 16
NCORES = 8
BL = B // NCORES  # batches per core: 4
TOK = BL * R      # 512 tokens per core per side
PTS = R * C       # 16384 score points per (b)

FP32 = mybir.dt.float32
FP16 = mybir.dt.float16
AF = mybir.ActivationFunctionType
ALU = mybir.AluOpType


def build_kernel():
    nc = bacc.Bacc("TRN2", target_bir_lowering=False, debug=False,
                   num_devices=NCORES)

    x_r = nc.dram_tensor("x_r", [TOK, E], FP32, kind="ExternalInput").ap()
    x_c = nc.dram_tensor("x_c", [TOK, E], FP32, kind="ExternalInput").ap()
    cost = nc.dram_tensor("cost", [BL, R, C], FP32, kind="ExternalInput").ap()
    wq_d = nc.dram_tensor("Wq", [E, E], FP32, kind="ExternalInput").ap()
    wk_d = nc.dram_tensor("Wk", [E, E], FP32, kind="ExternalInput").ap()
    wv_d = nc.dram_tensor("Wv", [E, E], FP32, kind="ExternalInput").ap()
    # layer1 stationary [17, 256]: col (half*128 + (h%8)*16 + m):
    #   row h' = a[h,m] iff h'==h; row 16 = c[h,m]
    w1_d = nc.dram_tensor("W1L", [17, 2 * 128], FP32,
                          kind="ExternalInput").ap()
    # layer2 moving [128, 16]: col (half*8 + j): row hm = w2[half*8+j, m]
    # iff hm == ((j)*16+m) else 0
    w2_d = nc.dram_tensor("W2L", [128, 16], FP32, kind="ExternalInput").ap()
    # relu bias per (h,m) row: bcol2[hm, half] = b1[half*8 + hm//16, hm%16]
    bc_d = nc.dram_tensor("bcol2", [128, 2], FP32, kind="ExternalInput").ap()
    out_d = nc.dram_tensor("out", [BL, R, H * D], FP32,
                           kind="ExternalOutput").ap()

    with tile.TileContext(nc) as tc, ExitStack() as ctx:
        const_p = ctx.enter_context(tc.tile_pool(name="const", bufs=1))
        stage_p = ctx.enter_context(tc.tile_pool(name="stage", bufs=1))
        xt_p = ctx.enter_context(tc.tile_pool(name="xt", bufs=1))
        w_p = ctx.enter_context(tc.tile_pool(name="wts", bufs=1))
        quad_p = ctx.enter_context(tc.tile_pool(name="quad", bufs=1))
        x4_p = ctx.enter_context(tc.tile_pool(name="x4", bufs=1))
        rhs_p = ctx.enter_context(tc.tile_pool(name="rhs", bufs=1))
        rr_p = ctx.enter_context(tc.tile_pool(name="rr", bufs=4))
        wsb_p = ctx.enter_context(tc.tile_pool(name="wsb", bufs=1))
        fout_p = ctx.enter_context(tc.tile_pool(name="fout", bufs=1))
        small_p = ctx.enter_context(tc.tile_pool(name="small", bufs=2))
        # PSUM (8 banks): ev 3x[128,1024] = 6 banks, l2 1x[128,512] = 1,
        # av 1x[128,144] = 1
        ps_ev = ctx.enter_context(
            tc.tile_pool(name="psev", bufs=3, space="PSUM"))
        ps_l2 = ctx.enter_context(
            tc.tile_pool(name="psl2", bufs=1, space="PSUM"))
        ps_av = ctx.enter_context(
            tc.tile_pool(name="psav", bufs=1, space="PSUM"))

        # round-robin assignment of setup evac work to the two PSUM engines
        eng_i = [0]

        def evac_engine():
            eng_i[0] ^= 1
            return nc.scalar if eng_i[0] else nc.vector

        def evac_copy(dst, src):
            ev = evac_engine()
            (ev.copy if ev is nc.scalar else ev.tensor_copy)(dst, src)

        # ---- input DMAs (SP queue): x absolutely first (the transpose
        # chain gates all of setup; HWDGE serializes at 625ns per DMA),
        # then q/k weights (needed at proj), everything else after ----
        x32 = {}
        for name, dram in (("r", x_r), ("c", x_c)):
            x32[name] = stage_p.tile([128, BL, E], FP32, tag=f"x32{name}",
                                     name=f"x32{name}")
            nc.sync.dma_start(
                x32[name][:], dram[:].rearrange("(t p) e -> p t e", p=128))
        xsl = {name: [x32[name][:, t, :] for t in range(BL)]
               for name in ("r", "c")}

        wdram = {"q": wq_d, "k": wk_d, "v": wv_d}

        def load_w32(name):
            w = stage_p.tile([128, 2, E], FP32, tag="w32", bufs=2,
                             name=f"w32{name}")
            nc.sync.dma_start(
                w[:], wdram[name][:].rearrange("(eh p) e -> p eh e", p=128))
            return w

        w32q = load_w32("q")
        w32k = load_w32("k")

        cost32 = stage_p.tile([128, BL, C], FP32, tag="cost32", name="cost32")
        nc.sync.dma_start(cost32[:], cost[:].rearrange("b r c -> r b c"))
        w1f = stage_p.tile([17, 256], FP32, tag="w1f", name="w1f")
        nc.sync.dma_start(w1f[:], w1_d[:])
        w2f = stage_p.tile([128, 16], FP32, tag="w2f", name="w2f")
        nc.sync.dma_start(w2f[:], w2_d[:])
        bcol2 = const_p.tile([128, 2], FP32)
        nc.sync.dma_start(bcol2[:], bc_d[:])

        # ---- const/weight prep ----
        ident = const_p.tile([128, 128], FP32)
        make_identity(nc, ident[:])

        w1l = const_p.tile([17, 256], FP16)
        nc.gpsimd.tensor_copy(w1l[:], w1f[:])
        w2l = const_p.tile([128, 16], FP16)
        nc.gpsimd.tensor_copy(w2l[:], w2f[:])

        # q/k weights padded on-chip: head h -> 32-col slot; v unpadded
        wt16 = {}
        for name, w32 in (("q", w32q), ("k", w32k)):
            wt = w_p.tile([128, 2, 2 * E], FP16, tag=f"wt{name}",
                          name=f"wt{name}")
            nc.gpsimd.memset(wt[:], 0.0)
            wt4 = wt[:].rearrange("p eh (h x) -> p eh h x", h=H)
            w4 = w32[:].rearrange("p eh (h d) -> p eh h d", h=H)
            nc.gpsimd.tensor_copy(wt4[:, :, :, 0:D], w4[:])
            wt16[name] = wt
        w32v = load_w32("v")
        wtv = w_p.tile([128, 2, E], FP16, tag="wtv", name="wtv")
        nc.gpsimd.tensor_copy(wtv[:], w32v[:])

        y16all = const_p.tile([128, BL * C], FP16, name="y16all")
        nc.gpsimd.tensor_copy(y16all[:], cost32[:].rearrange("p b c -> p (b c)"))

        # ---- PE transposes of the x slices ----
        xT = {}
        for name in ("r", "c"):
            xT[name] = [xt_p.tile([128, TOK], FP16, tag=f"xT{name}{eh}",
                                  name=f"xT{name}{eh}") for eh in range(2)]
            for eh in range(2):
                ps = ps_ev.tile([128, 1024], FP32, tag="ev", name="psev")
                for t in range(BL):
                    nc.tensor.transpose(
                        ps[:, t * 128:(t + 1) * 128],
                        xsl[name][t][:, eh * 128:(eh + 1) * 128], ident[:])
                evac_copy(xT[name][eh][:], ps[:, 0:512])

        # ---- projections + dots + gather, interleaved per quad-pair ----
        # (PE matmul operands must sit at base partition 0/32/64; 96 is
        #  invalid, so the 4th head of each quad lives in a base-0 tile)
        x4all = x4_p.tile([128, H * BL * C], FP16, name="x4all")
        x4v = x4all[:].rearrange("p (h b c) -> p h b c", h=H, b=BL)
        # single gathered rhs, b-major pts: [17, (b, r, c)] fp16.
        # One DMA per (head, b) keeps each DMA cheap in the tile scheduler's
        # internal (byte-cost-dominated) DMA model so later batches'
        # gathers aren't chained too deep behind main-loop progress.
        rhs_all = rhs_p.tile([17, BL * PTS], FP16, tag="rhs", name="rhs")

        def gather_h(h, b):
            nc.sync.dma_start(
                rhs_all[h:h + 1, b * PTS:(b + 1) * PTS], x4v[:, h, b, :])

        # head-GROUPS of 3 (heads at psum rows 0/32/64 -- all valid matmul
        # base partitions, unlike 96): 6 groups of (3,3,3,3,3,1) heads
        GSZ = [3, 3, 3, 3, 3, 1]
        GOF = [0, 3, 6, 9, 12, 15]
        qAll = {"q": quad_p.tile([96, 6 * TOK], FP16, tag="qAll",
                                 name="qAll"),
                "k": quad_p.tile([96, 6 * TOK], FP16, tag="kAll",
                                 name="kAll")}

        def proj_groups(proj, gp, dst):
            # two 3-head groups share one [128,1024] psum slot
            ps = ps_ev.tile([128, 2 * TOK], FP32, tag="ev", name="psev")
            for i in range(2):
                g = gp * 2 + i
                for eh in range(2):
                    nc.tensor.matmul(
                        ps[0:32 * GSZ[g], i * TOK:(i + 1) * TOK],
                        wt16[proj][:, eh, GOF[g] * 32:
                                   (GOF[g] + GSZ[g]) * 32],
                        xT["r" if proj == "q" else "c"][eh][:],
                        start=(eh == 0), stop=(eh == 1))
            for i in range(2):
                g = gp * 2 + i
                evac_copy(qAll[proj][0:32 * GSZ[g],
                                     g * TOK:(g + 1) * TOK],
                          ps[0:32 * GSZ[g], i * TOK:(i + 1) * TOK])

        def head_slice(proj, h):
            g, loc = h // 3, h % 3
            return qAll[proj][loc * 32:loc * 32 + 16,
                              g * TOK:(g + 1) * TOK]

        qT = [head_slice("q", h) for h in range(H)]
        kT = [head_slice("k", h) for h in range(H)]
        for gp in range(3):
            proj_groups("q", gp, None)
            proj_groups("k", gp, None)
            for hp in range(3 * gp, min(3 * gp + 3, 8)):  # head pairs
                ps = ps_ev.tile([128, 1024], FP32, tag="ev", name="psev")
                for i in range(2):
                    h = hp * 2 + i
                    for b in range(BL):
                        nc.tensor.matmul(
                            ps[:, i * 512 + b * 128:i * 512 + (b + 1) * 128],
                            qT[h][:, b * 128:(b + 1) * 128],
                            kT[h][:, b * 128:(b + 1) * 128])
                if hp < 6:
                    evac_copy(x4all[:, hp * 1024:(hp + 1) * 1024], ps[:])
                    gather_h(hp * 2, 0)
                    gather_h(hp * 2 + 1, 0)
                else:
                    # last pair: split the evac across both engines so the
                    # final b0 gather (the main-loop gate) fires earlier
                    nc.scalar.copy(x4all[:, hp * 1024:hp * 1024 + 512],
                                   ps[:, 0:512])
                    gather_h(hp * 2, 0)
                    nc.vector.tensor_copy(
                        x4all[:, hp * 1024 + 512:(hp + 1) * 1024],
                        ps[:, 512:1024])
                    gather_h(hp * 2 + 1, 0)
        # cost row b0, then remaining batches (overlap the main loop)
        nc.sync.dma_start(rhs_all[16:17, 0:PTS], y16all[:, 0:128])
        for b in range(1, BL):
            for h in range(H):
                gather_h(h, b)
            nc.sync.dma_start(
                rhs_all[16:17, b * PTS:(b + 1) * PTS],
                y16all[:, b * 128:(b + 1) * 128])

        # ---- v projections (first needed at AV(b0), well into main) ----
        vhat = []
        for b in range(BL):
            vh = quad_p.tile([128, 17 * H], FP32, tag=f"vhat{b}",
                             name=f"vhat{b}")
            vh3 = vh[:].rearrange("p (h x) -> p h x", h=H)
            nc.gpsimd.memset(vh3[:, :, 16:17], 1.0)
            ps = ps_ev.tile([128, 1024], FP32, tag="ev", name="psev")
            for eh in range(2):
                nc.tensor.matmul(
                    ps[:, 0:E], xT["c"][eh][:, b * 128:(b + 1) * 128],
                    wtv[:, eh, :], start=(eh == 0), stop=(eh == 1))
            evac_copy(vh3[:, :, 0:16],
                      ps[:, 0:E].rearrange("p (h d) -> p h d", h=H))
            vhat.append(vh)

        # ---- main loop: flat stream over 128 global chunks ----
        fouts = [fout_p.tile([128, H * D], FP32, tag=f"fo{b}", name=f"fo{b}")
                 for b in range(BL)]

        NIT = 2 * BL          # 8 (b, half) iterations
        NCK = 16              # chunks per iteration
        LAG = 2               # layer2 lag in chunks
        state = {}            # per-iteration: ps2, wsb, rr chunks

        def emit_av(it):
            b, half = it // 2, it % 2
            wsb = state[it]["wsb"]
            psa_t = ps_av.tile([128, 144], FP32, tag="av", name="psa_t")
            psa = psa_t[:, 0:17 * 8]
            wsb3 = wsb[:].rearrange("p (s h) -> p s h", h=8)
            for hl in range(8):
                h = half * 8 + hl
                nc.tensor.matmul(
                    psa[:, hl * 17:(hl + 1) * 17],
                    wsb3[:, :, hl],
                    vhat[b][:, h * 17:(h + 1) * 17])
            psa3 = psa.rearrange("p (x y) -> p x y", x=8)
            rec = small_p.tile([128, 8], FP32, tag="rec", name="rec")
            nc.vector.reciprocal(rec[:], psa3[:, :, 16])
            fo3 = fouts[b][:, half * 128:(half + 1) * 128].rearrange(
                "p (x y) -> p x y", x=8)
            nc.vector.tensor_tensor(
                fo3[:], psa3[:, :, 0:16],
                rec[:].broadcast_to((128, 8, 16)), ALU.mult)
            # ship each half as soon as it is normalized: the final
            # iteration's out-DMA then moves only 128 cols off the tail
            nc.sync.dma_start(
                out_d[b][:, half * 128:(half + 1) * 128],
                fouts[b][:, half * 128:(half + 1) * 128])

        def emit_l1(gck):
            it, ck = gck // NCK, gck % NCK
            b, half = it // 2, it % 2
            if ck == 0:
                state[it] = {
                    "wsb": wsb_p.tile([128, 1024], FP32, tag="wsb",
                                      name="wsb"),
                    "ps2": [None, None],
                    "rr": {},
                }
            ps = ps_ev.tile([128, 1024], FP32, tag="ev", name="psev")
            base = b * PTS + ck * 1024
            for j in range(2):
                nc.tensor.matmul(
                    ps[:, j * 512:(j + 1) * 512],
                    w1l[:, half * 128:(half + 1) * 128],
                    rhs_all[:, base + j * 512:base + (j + 1) * 512])
            rr = rr_p.tile([128, 1024], FP16, tag="rr", name="rr")
            to_act = ck % 2 == 0
            if to_act:
                nc.scalar.activation(rr[:], ps[:], AF.Relu,
                                     bias=bcol2[:, half:half + 1])
            else:
                nc.vector.tensor_scalar(rr[:], ps[:],
                                        bcol2[:, half:half + 1],
                                        0.0, ALU.add, ALU.max)
            state[it]["rr"][ck] = rr

        def emit_l2(gck):
            it, ck = gck // NCK, gck % NCK
            half = it % 2
            st = state[it]
            grp = ck // 8
            if ck % 8 == 0:
                st["ps2"][grp] = ps_l2.tile([128, 512], FP32, tag="l2",
                                           name="ps2")
            ps2 = st["ps2"][grp]
            rr = st["rr"].pop(ck)
            for s in range(8):
                rloc = (ck % 8) * 8 + s
                nc.tensor.matmul(
                    ps2[:, rloc * 8:rloc * 8 + 8],
                    rr[:, s * 128:(s + 1) * 128],
                    w2l[:, half * 8:(half + 1) * 8])
            if ck % 8 == 7:
                nc.scalar.activation(
                    st["wsb"][:, grp * 512:(grp + 1) * 512],
                    ps2[:], AF.Exp)

        TOTAL = NIT * NCK
        for gck in range(TOTAL + LAG):
            if gck < TOTAL:
                emit_l1(gck)
            if gck >= LAG:
                emit_l2(gck - LAG)
            if gck % NCK == 4 and gck // NCK >= 1 and gck < TOTAL:
                emit_av(gck // NCK - 1)
        emit_av(NIT - 1)

    nc.compile()
    return nc


_cache = {}


def kernel(**inputs):
    row_emb = np.asarray(inputs["row_emb"], dtype=np.float32)
    col_emb = np.asarray(inputs["col_emb"], dtype=np.float32)
    cost_mat = np.asarray(inputs["cost_mat"], dtype=np.float32)
    Wq = np.asarray(inputs["Wq"], dtype=np.float32)
    Wk = np.asarray(inputs["Wk"], dtype=np.float32)
    Wv = np.asarray(inputs["Wv"], dtype=np.float32)
    m1w = np.asarray(inputs["mix1_weight"], dtype=np.float32)
    m1b = np.asarray(inputs["mix1_bias"], dtype=np.float32)
    m2w = np.asarray(inputs["mix2_weight"], dtype=np.float32)

    a1 = m1w[:, 0, :]
    c1 = m1w[:, 1, :]
    w2 = m2w[:, :, 0]

    if "nc" not in _cache:
        _cache["nc"] = build_kernel()
    nc = _cache["nc"]

    wq_s = Wq * (1.0 / np.sqrt(D))

    w1l = np.zeros((17, 256), dtype=np.float32)
    w2l = np.zeros((128, 16), dtype=np.float32)
    bcol2 = np.zeros((128, 2), dtype=np.float32)
    for h in range(H):
        half, hl = h // 8, h % 8
        for m in range(MS):
            col = half * 128 + hl * 16 + m
            w1l[h, col] = a1[h, m]
            w1l[16, col] = c1[h, m]
            w2l[hl * 16 + m, half * 8 + hl] = w2[h, m]
            bcol2[hl * 16 + m, half] = m1b[h, m]

    in_maps = []
    for i in range(NCORES):
        sl = slice(i * BL, (i + 1) * BL)
        in_maps.append({
            "x_r": row_emb[sl].reshape(TOK, E),
            "x_c": col_emb[sl].reshape(TOK, E),
            "cost": cost_mat[sl],
            "Wq": wq_s, "Wk": Wk, "Wv": Wv,
            "W1L": w1l, "W2L": w2l, "bcol2": bcol2,
        })
    res = run_bass_kernel_spmd(nc, in_maps, list(range(NCORES)))
    out = np.concatenate([res.results[i]["out"] for i in range(NCORES)],
                         axis=0)
    return out.astype(np.float32)
